# revision 1
# baseline (speedup 1.0000x reference)
"""Trainium2 Bass kernel for a 2-layer TransformerConv GNN + MLP head.

Contract: kernel(**inputs) takes the FULL inputs (as produced by
setup_inputs()) and returns the FULL [N, 2] output, running the compute
on 8 NeuronCores via run_bass_kernel_spmd.

Sharding: nodes are padded to 50176 = 8 * 49 * 128 and split into 8
contiguous ranges of 49 node-tiles (128 nodes each). Each core owns the
edges whose *target* (dst) falls in its range (edge/data parallel with
disjoint segment sums -> no all-reduce needed). K/V node projections are
computed shard-wise and all-gathered so every core can gather arbitrary
source rows.

Edge pipeline per 128-node tile: per-edge rows of Q (by dst) and K|V
(by src) are fetched with dma_gather (int16 indices, tables split in two
halves to fit the int16 range); edge-attr projections e = ea @ We are
computed on the TensorEngine from host-transposed edge attributes; the
attention softmax is computed without max-subtraction (mathematically
identical, exp cannot overflow fp32 at these magnitudes); segment sums
over edges are one-hot matmuls into PSUM.
"""

import sys

sys.path.insert(0, "/opt/trn_rl_repo")

import os

import numpy as np
import ml_dtypes

import concourse.bacc as bacc
import concourse.bass as bass
import concourse.mybir as mybir
import concourse.tile as tile
from concourse.bass_utils import run_bass_kernel_spmd
from concourse.masks import make_identity

P = 128
NCORES = 8
FP = mybir.dt.float32

# problem dims (hardcoded per contract)
N_NODES = 50000
N_EDGES = 800000
F_NODE = 128
F_EDGE = 32
HEADS = 4
C1 = 32
C2 = 16
N_CLASSES = 2


# ----------------------------------------------------------------------------
# host-side preprocessing
# ----------------------------------------------------------------------------

def _wrap_idx(a):
    """[T, S] int16 -> dma_gather wrapped layout [T, 128, S//16]:
    index i of a call lands at [i % 16, i // 16], replicated x8 down
    the partitions (each GPSIMD core reads its own 16-partition group)."""
    T, S = a.shape
    w = np.ascontiguousarray(a.reshape(T, S // 16, 16).transpose(0, 2, 1))
    return np.tile(w, (1, 8, 1))


def host_prep(x, edge_index, edge_attr, n_nodes, n_edges, fe):
    """Build per-core device inputs for the edge phase."""
    t_total = -(-n_nodes // P)                      # ceil
    t_core = -(-t_total // NCORES)
    t_all = t_core * NCORES
    n_pad = t_all * P
    n_core = t_core * P
    half = (n_pad // 2 + P - 1) // P * P            # split point for int16 tables
    assert half < 32768 and n_pad - half < 32768

    src = np.asarray(edge_index[0], dtype=np.int64)
    dst = np.asarray(edge_index[1], dtype=np.int64)
    ea = np.asarray(edge_attr, dtype=np.float32)

    tile_of = dst // P
    key = (tile_of * 2 + (src >= half)).astype(np.int64)
    order = np.argsort(key, kind="stable")
    counts = np.bincount(key, minlength=t_all * 2)
    cl = int(-(-counts[0::2].max() // P))           # lo chunks per tile
    ch = int(-(-counts[1::2].max() // P))           # hi chunks per tile
    ct = cl + ch
    cap = ct * P

    sorted_keys = key[order]
    grp_starts = np.concatenate(([0], np.cumsum(counts)[:-1]))
    pos = np.arange(n_edges) - grp_starts[sorted_keys]
    dest = (sorted_keys // 2) * cap + (sorted_keys % 2) * (cl * P) + pos

    slot_edge = np.full(t_all * cap, -1, np.int64)
    slot_edge[dest] = order
    valid = slot_edge >= 0
    e_idx = np.where(valid, slot_edge, 0)
    src_s = src[e_idx]
    dst_s = dst[e_idx]
    t_arr = np.repeat(np.arange(t_all), cap)

    kvidx = np.where(valid, np.where(src_s < half, src_s, src_s - half), 0)
    kvidx = kvidx.astype(np.int16).reshape(t_all, cap)
    core_base = (t_arr // t_core) * n_core
    qidx = np.where(valid, dst_s - core_base, 0).astype(np.int16).reshape(t_all, cap)
    dstrel = np.where(valid, dst_s - t_arr * P, -1).astype(ml_dtypes.bfloat16)
    dstrel = dstrel.reshape(t_all, ct, P)            # [T, chunk, edge-in-chunk]
    ea_slots = np.where(valid[:, None], ea[e_idx], 0).astype(np.float32)
    eaT = np.ascontiguousarray(
        ea_slots.reshape(t_all, cap, fe).transpose(0, 2, 1)
    )                                               # [T, FE, cap]

    # per-section wrapped gather indices, concatenated: [T, 128, ct*8]
    kvw = np.concatenate(
        [_wrap_idx(kvidx[:, : cl * P]), _wrap_idx(kvidx[:, cl * P:])], axis=2
    )
    qw = np.concatenate(
        [_wrap_idx(qidx[:, : cl * P]), _wrap_idx(qidx[:, cl * P:])], axis=2
    )
    # dstrel laid out [T, 128, ct] (partition = edge-in-chunk)
    dstrel_t = np.ascontiguousarray(dstrel.transpose(0, 2, 1))

    x_pad = np.zeros((n_pad, x.shape[1]), np.float32)
    x_pad[:n_nodes] = x

    percore = []
    for c in range(NCORES):
        ts = slice(c * t_core, (c + 1) * t_core)
        percore.append(
            dict(
                xT=np.ascontiguousarray(x_pad[c * n_core:(c + 1) * n_core].T),
                eaT=np.ascontiguousarray(eaT[ts]),
                kvidx=np.ascontiguousarray(
                    kvw[ts].transpose(1, 0, 2).reshape(P, -1)),
                qidx=np.ascontiguousarray(
                    qw[ts].transpose(1, 0, 2).reshape(P, -1)),
                dstrel=np.ascontiguousarray(
                    dstrel_t[ts].transpose(1, 0, 2).reshape(P, -1)),
            )
        )
    dcfg = dict(
        t_core=t_core, cl=cl, ch=ch, half=half, n_pad=n_pad, n_core=n_core,
        fn=x.shape[1], fe=fe, h=HEADS, c1=C1, c2=C2, ncls=N_CLASSES,
    )
    return percore, dcfg


# ----------------------------------------------------------------------------
# device program
# ----------------------------------------------------------------------------

def _edge_layer(nc, tc, pool, psum, cfg, consts, layer):
    """One TransformerConv edge pass over this core's tiles.

    Gathers per-edge Q (by dst) and K|V (by src) rows, computes the edge
    softmax without max-subtraction, and accumulates one-hot segment-sum
    matmuls into PSUM. Epilogues are batched over TG-tile groups; the
    relu'd per-node result lands in layer["h_res"] ([128, t_core*c]).
    """
    t_core, cl, ch = cfg["t_core"], cfg["cl"], cfg["ch"]
    ct = cl + ch
    half, fe, H = cfg["half"], cfg["fe"], cfg["h"]
    c = layer["c"]
    hc = H * c
    iota = consts["iota"]
    kvidx_sb, qidx_sb, dstrel_sb = consts["kvidx"], consts["qidx"], consts["dstrel"]
    scale = 1.0 / float(np.sqrt(c))

    q_dram, kv_full = layer["q_dram"], layer["kv_full"]
    q_step = layer["q_step"]
    We_sb = layer["We_sb"]
    h_res = layer["h_res"]
    G = 6                                            # chunks per DVE slab group
    groups = [(g, min(G, ct - g)) for g in range(0, ct, G)]
    TG = 8                                           # tiles per epilogue batch
    MAXC = 8                     # dma_gather tops out at 1024 indices/call

    # skip connection rows for all own tiles, resident: [128, t_core*c]
    skip_all = layer["pool1"].tile([P, t_core * c], FP, tag="skip_all")
    nc.scalar.dma_start(
        out=skip_all[:].rearrange("p (t w) -> p t w", t=t_core),
        in_=q_dram[:, hc:hc + c].rearrange("(t p) w -> p t w", p=P))

    agg_grp = None
    for t in range(t_core):
        deng = nc.sync if t % 2 == 0 else nc.scalar
        eaT_t = pool.tile([fe, ct * P], FP, tag="eaT")
        deng.dma_start(out=eaT_t[:], in_=layer["eaT_dram"][t])

        q_e = pool.tile([P, ct, hc], FP, tag="q_e")
        kv_e = pool.tile([P, ct, 2 * hc], FP, tag="kv_e")
        if t < 2:
            nc.vector.memset(q_e[:], 0.0)
            nc.vector.memset(kv_e[:], 0.0)
        qi = qidx_sb[:, t * ct * 8:(t + 1) * ct * 8]
        ki = kvidx_sb[:, t * ct * 8:(t + 1) * ct * 8]

        def emit_gathers(out_tile, table_ap, idx_ap, c0, nch, elem, step=None,
                         queue=0):
            for s0 in range(0, nch, MAXC):
                n = min(MAXC, nch - s0)
                nc.gpsimd.dma_gather(
                    out_tile[:, c0 + s0:c0 + s0 + n, :], table_ap,
                    idx_ap[:, (c0 + s0) * 8:(c0 + s0 + n) * 8],
                    n * P, n * P, elem, elem_step=step, queue_num=queue)

        nq = int(os.environ.get("KBUILD_NQ", "4"))
        emit_gathers(q_e, q_dram[:, 0:hc], qi, 0, cl, hc, q_step,
                     queue=1 % nq)
        emit_gathers(q_e, q_dram[:, 0:hc], qi, cl, ch, hc, q_step,
                     queue=3 % nq)
        emit_gathers(kv_e, kv_full[:half, :], ki, 0, cl, 2 * hc, queue=0)
        emit_gathers(kv_e, kv_full[half:, :], ki, cl, ch, 2 * hc,
                     queue=2 % nq)

        agg_ps = psum.tile([P, H * (c + 1)], FP, space="PSUM", tag="agg")
        first = True
        for g0, gn in groups:
            e_ps = psum.tile([P, G * hc], FP, space="PSUM", tag="e_ps")
            for j in range(gn):
                nc.tensor.matmul(
                    out=e_ps[:, j * hc:(j + 1) * hc],
                    lhsT=eaT_t[:, (g0 + j) * P:(g0 + j + 1) * P],
                    rhs=We_sb[:],
                    start=True, stop=True,
                )
            e_v = e_ps[:].rearrange("p (g f) -> p g f", g=G)[:, 0:gn, :]
            ke = pool.tile([P, G * hc], FP, tag="ke")
            ve = pool.tile([P, G * hc], mybir.dt.bfloat16, tag="ve")
            nc.vector.tensor_tensor(
                out=ke[:].rearrange("p (g f) -> p g f", g=G)[:, 0:gn, :],
                in0=kv_e[:, g0:g0 + gn, 0:hc], in1=e_v, op=mybir.AluOpType.add)
            nc.vector.tensor_tensor(
                out=ve[:].rearrange("p (g f) -> p g f", g=G)[:, 0:gn, :],
                in0=kv_e[:, g0:g0 + gn, hc:2 * hc], in1=e_v,
                op=mybir.AluOpType.add)
            nc.vector.tensor_tensor(
                out=ke[:].rearrange("p (g f) -> p g f", g=G)[:, 0:gn, :],
                in0=q_e[:, g0:g0 + gn, :],
                in1=ke[:].rearrange("p (g f) -> p g f", g=G)[:, 0:gn, :],
                op=mybir.AluOpType.mult)
            lg = pool.tile([P, G * H], FP, tag="lg")
            nc.vector.reduce_sum(
                out=lg[:].rearrange("p (g h) -> p g h", g=G)[:, 0:gn, :],
                in_=ke[:].rearrange("p (g h w) -> p g h w", g=G, h=H)[:, 0:gn],
                axis=mybir.AxisListType.X)
            p_t = pool.tile([P, G * H], mybir.dt.bfloat16, tag="p_t")
            nc.scalar.activation(
                out=p_t[:, 0:gn * H], in_=lg[:, 0:gn * H],
                func=mybir.ActivationFunctionType.Exp, scale=scale)
            pv = pool.tile([P, G * H * (c + 1)], mybir.dt.bfloat16, tag="pv")
            pv4 = pv[:].rearrange("p (g h w) -> p g h w", g=G, h=H)
            p3 = p_t[:].rearrange("p (g h) -> p g h", g=G)
            nc.vector.tensor_tensor(
                out=pv4[:, 0:gn, :, 0:c],
                in0=ve[:].rearrange("p (g h w) -> p g h w", g=G, h=H)[:, 0:gn],
                in1=p3[:, 0:gn, :, None].to_broadcast([P, gn, H, c]),
                op=mybir.AluOpType.mult)
            nc.vector.tensor_copy(out=pv4[:, 0:gn, :, c], in_=p3[:, 0:gn, :])
            oh = pool.tile([P, G * P], mybir.dt.bfloat16, tag="oh")
            nc.vector.tensor_tensor(
                out=oh[:].rearrange("p (g f) -> p g f", g=G)[:, 0:gn, :],
                in0=iota[:].rearrange("p (g f) -> p g f", g=G)[:, 0:gn, :],
                in1=dstrel_sb[:, t * ct + g0: t * ct + g0 + gn][:, :, None]
                    .to_broadcast([P, gn, P]),
                op=mybir.AluOpType.is_equal)
            for j in range(gn):
                nc.tensor.matmul(
                    out=agg_ps[:],
                    lhsT=oh[:, j * P:(j + 1) * P],
                    rhs=pv[:, j * H * (c + 1):(j + 1) * H * (c + 1)],
                    start=first, stop=(g0 + j == ct - 1),
                )
                first = False

        # stash this tile's PSUM aggregate; epilogues run batched per TG tiles
        tg = t % TG
        if tg == 0:
            agg_grp = pool.tile([P, TG * H * (c + 1)], FP, tag="agg_grp")
        nc.vector.tensor_copy(
            out=agg_grp[:, tg * H * (c + 1):(tg + 1) * H * (c + 1)],
            in_=agg_ps[:])
        if tg == TG - 1 or t == t_core - 1:
            n = tg + 1
            t0 = t - tg
            a4 = agg_grp[:].rearrange("p (t h w) -> p t h w", t=TG, h=H)
            sp = pool.tile([P, TG * H], FP, tag="sp")
            nc.vector.tensor_scalar(
                out=sp[:, 0:n * H],
                in0=a4[:, 0:n, :, c].rearrange("p t h -> p (t h)"),
                scalar1=1e-30, scalar2=None, op0=mybir.AluOpType.add)
            rs = pool.tile([P, TG * H], FP, tag="rs")
            nc.vector.reciprocal(out=rs[:, 0:n * H], in_=sp[:, 0:n * H])
            nc.vector.tensor_scalar(
                out=rs[:, 0:n * H], in0=rs[:, 0:n * H], scalar1=1.0 / H,
                scalar2=None, op0=mybir.AluOpType.mult)
            nc.vector.tensor_tensor(
                out=a4[:, 0:n, :, 0:c], in0=a4[:, 0:n, :, 0:c],
                in1=rs[:].rearrange("p (t h) -> p t h", t=TG)[:, 0:n, :, None]
                    .to_broadcast([P, n, H, c]),
                op=mybir.AluOpType.mult)
            hsum = pool.tile([P, TG * c], FP, tag="hsum")
            nc.vector.reduce_sum(
                out=hsum[:].rearrange("p (t w) -> p t w", t=TG)[:, 0:n],
                in_=agg_grp[:].rearrange("p (t h w) -> p t w h", t=TG,
                                         h=H)[:, 0:n, 0:c, :],
                axis=mybir.AxisListType.X)
            nc.vector.tensor_tensor(
                out=hsum[:, 0:n * c], in0=hsum[:, 0:n * c],
                in1=skip_all[:, t0 * c:(t0 + n) * c],
                op=mybir.AluOpType.add)
            nc.scalar.activation(
                out=h_res[:, t0 * c:(t0 + n) * c], in_=hsum[:, 0:n * c],
                func=mybir.ActivationFunctionType.Relu)


def build_device(dcfg):
    phases = os.environ.get("KBUILD_PHASES", "F")
    t_core, cl, ch = dcfg["t_core"], dcfg["cl"], dcfg["ch"]
    ct = cl + ch
    n_pad, n_core = dcfg["n_pad"], dcfg["n_core"]
    fn, fe, H = dcfg["fn"], dcfg["fe"], dcfg["h"]
    c1, c2, ncls = dcfg["c1"], dcfg["c2"], dcfg["ncls"]
    hc1, hc2 = H * c1, H * c2
    hid = 2 * c2

    nc = bacc.Bacc("TRN2", target_bir_lowering=False, debug=False,
                   num_devices=NCORES, num_swdge_queues=4)

    def param(name, shape, dtype=FP, out=False):
        return nc.declare_dram_parameter(name, list(shape), dtype, isOutput=out)

    xT_d = param("xT", [fn, n_core])
    eaT_d = param("eaT", [t_core, fe, ct * P])
    kvidx_d = param("kvidx", [P, t_core * ct * 8], mybir.dt.int16)
    qidx_d = param("qidx", [P, t_core * ct * 8], mybir.dt.int16)
    dstrel_d = param("dstrel", [P, t_core * ct], mybir.dt.bfloat16)
    wkv1_d = param("wkv1", [fn, 2 * hc1])
    bkv1_d = param("bkv1", [1, 2 * hc1])
    wqs1_d = param("wqs1", [fn, hc1 + c1])
    bqs1_d = param("bqs1", [1, hc1 + c1])
    we1_d = param("we1", [fe, hc1])
    wkv2_d = param("wkv2", [c1, 2 * hc2])
    bkv2_d = param("bkv2", [1, 2 * hc2])
    wqs2_d = param("wqs2", [c1, hc2 + c2])
    bqs2_d = param("bqs2", [1, hc2 + c2])
    we2_d = param("we2", [fe, hc2])
    w3_d = param("w3", [c2, hid])
    b3_d = param("b3", [hid, 1])
    w4_d = param("w4", [hid, ncls])
    b4_d = param("b4", [ncls, 1])
    out_d = param("out", [ncls, n_core], out=True)

    with tile.TileContext(nc) as tc:
        with (
            tc.tile_pool(name="res", bufs=1) as res,
            tc.tile_pool(name="sbuf", bufs=2) as pool,
            tc.tile_pool(name="sbuf1", bufs=1) as pool1,
            tc.tile_pool(name="dram", bufs=1, space="DRAM") as dram,
        ):
            # ---- constants / resident tensors
            ident = res.tile([P, P], FP)
            make_identity(nc, ident[:])
            ones_row = res.tile([1, P], FP)
            nc.vector.memset(ones_row[:], 1.0)
            iota = res.tile([P, 6 * P], mybir.dt.bfloat16)
            nc.gpsimd.iota(iota[:, 0:P], pattern=[[1, P]], base=0,
                           channel_multiplier=0,
                           allow_small_or_imprecise_dtypes=True)
            for g in range(1, 6):
                nc.vector.tensor_copy(out=iota[:, g * P:(g + 1) * P],
                                      in_=iota[:, 0:P])
            kvidx_sb = res.tile([P, t_core * ct * 8], mybir.dt.int16)
            nc.sync.dma_start(out=kvidx_sb[:], in_=kvidx_d[:])
            qidx_sb = res.tile([P, t_core * ct * 8], mybir.dt.int16)
            nc.sync.dma_start(out=qidx_sb[:], in_=qidx_d[:])
            dstrel_sb = res.tile([P, t_core * ct], mybir.dt.bfloat16)
            nc.sync.dma_start(out=dstrel_sb[:], in_=dstrel_d[:])

            def load_w(d, shape, tag, dt=FP):
                t = res.tile(list(shape), dt, tag=tag)
                nc.sync.dma_start(out=t[:], in_=d[:])
                return t

            wkv1 = load_w(wkv1_d, [fn, 2 * hc1], "wkv1")
            bkv1 = load_w(bkv1_d, [1, 2 * hc1], "bkv1")
            wqs1 = load_w(wqs1_d, [fn, hc1 + c1], "wqs1")
            bqs1 = load_w(bqs1_d, [1, hc1 + c1], "bqs1")
            we1 = load_w(we1_d, [fe, hc1], "we1")
            wkv2 = load_w(wkv2_d, [c1, 2 * hc2], "wkv2")
            bkv2 = load_w(bkv2_d, [1, 2 * hc2], "bkv2")
            wqs2 = load_w(wqs2_d, [c1, hc2 + c2], "wqs2")
            bqs2 = load_w(bqs2_d, [1, hc2 + c2], "bqs2")
            we2 = load_w(we2_d, [fe, hc2], "we2")
            w3 = load_w(w3_d, [c2, hid], "w3")
            b3 = load_w(b3_d, [hid, 1], "b3")
            w4 = load_w(w4_d, [hid, ncls], "w4")
            b4 = load_w(b4_d, [ncls, 1], "b4")

            h1_res = res.tile([P, t_core * c1], FP)
            h2_res = res.tile([P, t_core * c2], FP)
            h2T_res = res.tile([c2, t_core * P], FP)

            # ---- internal DRAM
            kv1_shard = dram.tile([n_core, 2 * hc1], FP)
            kv1_full = dram.tile([n_pad, 2 * hc1], FP)
            qs1_dram = dram.tile([n_core, 192], FP)
            kv2_shard = dram.tile([n_core, 2 * hc2], FP)
            kv2_full = dram.tile([n_pad, 2 * hc2], FP)
            qs2_dram = dram.tile([n_core, 128], FP)

            reps = int(os.environ.get("KBUILD_REPS", "1"))

            def emit_pipeline():
                # ---- phase A: layer-1 projections for own node range
                with tc.tile_pool(name="psumA", bufs=2, space="PSUM") as psum:
                  for t in range(t_core):
                      deng = nc.sync if t % 2 == 0 else nc.scalar
                      xT_t = pool.tile([fn, P], FP, tag="xT_t")
                      deng.dma_start(out=xT_t[:], in_=xT_d[:, t * P:(t + 1) * P])
                      pr_ps = psum.tile([P, 2 * hc1 + hc1 + c1], FP, space="PSUM",
                                        tag="pr_ps")
                      nc.tensor.matmul(out=pr_ps[:, 0:2 * hc1], lhsT=xT_t[:],
                                       rhs=wkv1[:], start=True, stop=False)
                      nc.tensor.matmul(out=pr_ps[:, 0:2 * hc1], lhsT=ones_row[:1, :],
                                       rhs=bkv1[:1, :], start=False, stop=True)
                      nc.tensor.matmul(out=pr_ps[:, 2 * hc1:], lhsT=xT_t[:],
                                       rhs=wqs1[:], start=True, stop=False)
                      nc.tensor.matmul(out=pr_ps[:, 2 * hc1:], lhsT=ones_row[:1, :],
                                       rhs=bqs1[:1, :], start=False, stop=True)
                      pr_sb = pool.tile([P, 2 * hc1 + hc1 + c1], FP, tag="pr_sb")
                      nc.vector.tensor_copy(out=pr_sb[:], in_=pr_ps[:])
                      deng.dma_start(out=kv1_shard[t * P:(t + 1) * P, :],
                                     in_=pr_sb[:, 0:2 * hc1])
                      deng.dma_start(out=qs1_dram[t * P:(t + 1) * P, 0:hc1 + c1],
                                     in_=pr_sb[:, 2 * hc1:])

                if phases >= "AG":
                    nc.gpsimd.collective_compute(
                        "AllGather", mybir.AluOpType.bypass,
                        replica_groups=[list(range(NCORES))],
                        ins=[kv1_shard[:].opt()], outs=[kv1_full[:].opt()])

                consts = dict(iota=iota, kvidx=kvidx_sb, qidx=qidx_sb,
                              dstrel=dstrel_sb)

                # ---- phase B: layer-1 edge pass
                if phases < "B":
                    nc.vector.memset(h1_res[:], 0.0)

                if phases >= "B":
                  with tc.tile_pool(name="psumB", bufs=2, space="PSUM") as psum:
                    _edge_layer(nc, tc, pool, psum, dcfg, consts, dict(
                        c=c1, q_dram=qs1_dram, q_step=192, kv_full=kv1_full,
                        We_sb=we1, eaT_dram=eaT_d,
                        h_res=h1_res[:], pool1=pool1))

                # ---- phase D: layer-2 projections from h1 (own range, resident)
                if phases >= "D":
                 with tc.tile_pool(name="psumD", bufs=2, space="PSUM") as psum:
                  for t in range(t_core):
                      h1T_ps = psum.tile([c1, P], FP, space="PSUM", tag="h1T_ps")
                      nc.tensor.transpose(
                          out=h1T_ps[:], in_=h1_res[:, t * c1:(t + 1) * c1],
                          identity=ident[:])
                      h1T = pool.tile([c1, P], FP, tag="h1T")
                      nc.vector.tensor_copy(out=h1T[:], in_=h1T_ps[:])
                      p2_ps = psum.tile([P, 2 * hc2 + hc2 + c2], FP, space="PSUM",
                                        tag="p2_ps")
                      nc.tensor.matmul(out=p2_ps[:, 0:2 * hc2], lhsT=h1T[:],
                                       rhs=wkv2[:], start=True, stop=False)
                      nc.tensor.matmul(out=p2_ps[:, 0:2 * hc2], lhsT=ones_row[:1, :],
                                       rhs=bkv2[:1, :], start=False, stop=True)
                      nc.tensor.matmul(out=p2_ps[:, 2 * hc2:], lhsT=h1T[:],
                                       rhs=wqs2[:], start=True, stop=False)
                      nc.tensor.matmul(out=p2_ps[:, 2 * hc2:], lhsT=ones_row[:1, :],
                                       rhs=bqs2[:1, :], start=False, stop=True)
                      p2_sb = pool.tile([P, 2 * hc2 + hc2 + c2], FP, tag="p2_sb")
                      nc.vector.tensor_copy(out=p2_sb[:], in_=p2_ps[:])
                      deng = nc.sync if t % 2 == 0 else nc.scalar
                      deng.dma_start(out=kv2_shard[t * P:(t + 1) * P, :],
                                     in_=p2_sb[:, 0:2 * hc2])
                      deng.dma_start(out=qs2_dram[t * P:(t + 1) * P, 0:hc2 + c2],
                                     in_=p2_sb[:, 2 * hc2:])

                if phases >= "D":
                    nc.gpsimd.collective_compute(
                        "AllGather", mybir.AluOpType.bypass,
                        replica_groups=[list(range(NCORES))],
                        ins=[kv2_shard[:].opt()], outs=[kv2_full[:].opt()])

                # ---- phase E: layer-2 edge pass
                if phases >= "E":
                  with tc.tile_pool(name="psumE", bufs=2, space="PSUM") as psum:
                    _edge_layer(nc, tc, pool, psum, dcfg, consts, dict(
                        c=c2, q_dram=qs2_dram, q_step=128, kv_full=kv2_full,
                        We_sb=we2, eaT_dram=eaT_d,
                        h_res=h2_res[:], pool1=pool1))
                    for t in range(t_core):
                        h2T_ps = psum.tile([c2, P], FP, space="PSUM",
                                           tag="h2T_ps")
                        nc.tensor.transpose(
                            out=h2T_ps[:], in_=h2_res[:, t * c2:(t + 1) * c2],
                            identity=ident[:])
                        nc.vector.tensor_copy(
                            out=h2T_res[:, t * P:(t + 1) * P], in_=h2T_ps[:])

                # ---- phase F: dense head on h2T (outputs transposed [ncls, n_core])
                CHUNK = 512
                if phases < "E":
                    nc.vector.memset(h2T_res[:], 0.0)
                with tc.tile_pool(name="psumF", bufs=2, space="PSUM") as psum:
                  for k0 in range(0, n_core, CHUNK):
                      kn = min(CHUNK, n_core - k0)
                      h3_ps = psum.tile([hid, CHUNK], FP, space="PSUM", tag="h3_ps")
                      nc.tensor.matmul(out=h3_ps[:, 0:kn], lhsT=w3[:],
                                       rhs=h2T_res[:, k0:k0 + kn], start=True,
                                       stop=True)
                      h3_sb = pool.tile([hid, CHUNK], FP, tag="h3_sb")
                      nc.scalar.activation(
                          out=h3_sb[:, 0:kn], in_=h3_ps[:, 0:kn],
                          func=mybir.ActivationFunctionType.Relu, bias=b3[:, 0:1])
                      o_ps = psum.tile([ncls, CHUNK], FP, space="PSUM", tag="o_ps")
                      nc.tensor.matmul(out=o_ps[:, 0:kn], lhsT=w4[:],
                                       rhs=h3_sb[:, 0:kn], start=True, stop=True)
                      o_sb = pool.tile([ncls, CHUNK], FP, tag="o_sb")
                      nc.vector.tensor_scalar(
                          out=o_sb[:, 0:kn], in0=o_ps[:, 0:kn], scalar1=b4[:, 0:1],
                          scalar2=None, op0=mybir.AluOpType.add)
                      nc.sync.dma_start(out=out_d[:, k0:k0 + kn], in_=o_sb[:, 0:kn])


            for _rep in range(reps):
                emit_pipeline()

    nc.compile()
    return nc


# ----------------------------------------------------------------------------
# entry point
# ----------------------------------------------------------------------------

_CACHE = {}


def _get_nc(dcfg):
    key = tuple(sorted(dcfg.items()))
    if key not in _CACHE:
        _CACHE[key] = build_device(dcfg)
    return _CACHE[key]


def kernel(x, edge_index, edge_attr,
           Wq1, bq1, Wk1, bk1, Wv1, bv1, We1, Ws1, bs1,
           Wq2, bq2, Wk2, bk2, Wv2, bv2, We2, Ws2, bs2,
           W3, b3, W4, b4):
    x = np.asarray(x, np.float32)
    n_nodes = x.shape[0]
    n_edges = np.asarray(edge_index).shape[1]
    percore, dcfg = host_prep(x, np.asarray(edge_index),
                              np.asarray(edge_attr, np.float32),
                              n_nodes, n_edges, np.asarray(edge_attr).shape[1])
    f32 = lambda a: np.ascontiguousarray(np.asarray(a, np.float32))
    weights = dict(
        wkv1=np.concatenate([f32(Wk1), f32(Wv1)], axis=1),
        bkv1=np.concatenate([f32(bk1), f32(bv1)])[None, :],
        wqs1=np.concatenate([f32(Wq1), f32(Ws1)], axis=1),
        bqs1=np.concatenate([f32(bq1), f32(bs1)])[None, :],
        we1=f32(We1),
        wkv2=np.concatenate([f32(Wk2), f32(Wv2)], axis=1),
        bkv2=np.concatenate([f32(bk2), f32(bv2)])[None, :],
        wqs2=np.concatenate([f32(Wq2), f32(Ws2)], axis=1),
        bqs2=np.concatenate([f32(bq2), f32(bs2)])[None, :],
        we2=f32(We2),
        w3=f32(W3), b3=f32(b3)[:, None],
        w4=f32(W4), b4=f32(b4)[:, None],
    )
    in_maps = [dict(pc, **weights) for pc in percore]
    nc = _get_nc(dcfg)
    res = run_bass_kernel_spmd(nc, in_maps, core_ids=list(range(NCORES)))
    out = np.concatenate([res.results[i]["out"].T for i in range(NCORES)])
    return np.ascontiguousarray(out[:n_nodes])



# revision 35
# speedup vs baseline: 10.4149x; 10.4149x over previous
"""Trainium2 Bass kernel for a 2-layer TransformerConv GNN + MLP head.

Contract: kernel(**inputs) takes the FULL inputs (as produced by
setup_inputs()) and returns the FULL [N, 2] output, running the compute
on 8 NeuronCores via run_bass_kernel_spmd.

Sharding: nodes are padded to 50176 = 8 * 49 * 128 and split into 8
contiguous ranges of 49 node-tiles (128 nodes each). Each core owns the
edges whose *target* (dst) falls in its range (edge/data parallel with
disjoint segment sums -> no all-reduce needed).

v2 design notes:
- No kv1 AllGather: x is replicated (bf16, transposed); every core
  computes the FULL kv1 projection table redundantly (PE is idle) and
  writes it to local DRAM. The only collective is a small AllGather of
  per-tile-transposed h1 (bf16, 3.2MB total) between the layers.
- Gather tables are bf16 (kv rows 512B / 256B) to halve gather DMA.
- Bias algebra: bk is dropped (adding q.bk to all logits of one dst is
  softmax-invariant -> exactly no effect on alpha); bv is folded into
  the skip bias on the host (sum_e alpha = 1); bq/bs ride the q/skip
  projection of the 49 own tiles only.
- Edge pipeline per 128-node tile: per-edge Q (by dst) and K|V (by src)
  rows fetched with dma_gather (int16 indices; kv table split in two
  halves for the int16 range; gather calls span PAIRS of tiles to halve
  the 994ns/call SWDGE prep); e = ea @ We on the PE from bf16
  transposed edge attributes; softmax without max-subtraction
  (mathematically identical here; exp cannot overflow fp32 at these
  magnitudes); segment sums over edges are one-hot matmuls into PSUM.
- DVE hot path is all-SBUF bf16 (4x/2x DVE perf modes); the PSUM->SBUF
  copy of e runs on the Activation engine.
"""

import sys

sys.path.insert(0, "/opt/trn_rl_repo")

import os

import numpy as np
import ml_dtypes

import concourse.bacc as bacc
import concourse.bass as bass
import concourse.mybir as mybir
import concourse.tile as tile
from concourse.bass_utils import run_bass_kernel_spmd
from concourse.masks import make_identity

P = 128
NCORES = 8
FP = mybir.dt.float32
BF = mybir.dt.bfloat16
GS = 2                      # tiles per gather-call group

# problem dims (hardcoded per contract)
N_NODES = 50000
N_EDGES = 800000
F_NODE = 128
F_EDGE = 32
HEADS = 4
C1 = 32
C2 = 16
N_CLASSES = 2


# ----------------------------------------------------------------------------
# host-side preprocessing
# ----------------------------------------------------------------------------

def _wrap_idx(a):
    """[T, S] int16 -> dma_gather wrapped layout [T, 128, S//16]:
    index i of a call lands at [i % 16, i // 16], replicated x8 down
    the partitions (each GPSIMD core reads its own 16-partition group)."""
    T, S = a.shape
    w = np.ascontiguousarray(a.reshape(T, S // 16, 16).transpose(0, 2, 1))
    return np.tile(w, (1, 8, 1))


def host_prep(x, edge_index, edge_attr, n_nodes, n_edges, fe):
    """Build per-core device inputs for the edge phase.

    Slot layout per PAIR of consecutive tiles (t0=2p, t1=2p+1):
      [lo(t0) cl chunks | lo(t1) cl | hi(t0) ch | hi(t1) ch]
    so one gather call covers the lo (resp. hi) sections of both tiles.
    """
    t_total = -(-n_nodes // P)                      # ceil
    t_core = -(-t_total // NCORES)
    if t_core % GS:
        t_core += GS - t_core % GS
    t_all = t_core * NCORES
    n_pad = t_all * P
    n_core = t_core * P
    half = (n_pad // 2 + P - 1) // P * P            # split point for int16 tables
    assert half < 32768 and n_pad - half < 32768 and n_core < 32768

    src = np.asarray(edge_index[0], dtype=np.int64)
    dst = np.asarray(edge_index[1], dtype=np.int64)
    ea = np.asarray(edge_attr, dtype=np.float32)

    tile_of = dst // P
    hi = (src >= half).astype(np.int64)
    # group = pair index; section within pair = hi*2 + (tile parity)
    key = (tile_of // GS) * 2 * GS + hi * GS + (tile_of % GS)
    order = np.lexsort((dst, key))      # dst-sorted within each section
    counts = np.bincount(key, minlength=t_all * 2)
    # per (tile, half) chunk capacity, uniform across all tiles
    cnt2 = counts.reshape(t_all // GS, 2, GS)
    cl = int(-(-cnt2[:, 0, :].max() // P))          # lo chunks per tile
    ch = int(-(-cnt2[:, 1, :].max() // P))          # hi chunks per tile
    ct = cl + ch
    cap = ct * P                                    # slots per tile
    paircap = GS * cap

    # destination slot of each edge, in pair-call order
    # key % (2*GS): hi*GS + parity ; order lo(t0),lo(t1),hi(t0),hi(t1)
    sec_sizes = np.array([cl * P] * GS + [ch * P] * GS)
    sec_starts = np.concatenate(([0], np.cumsum(sec_sizes)[:-1]))
    grp_starts = np.concatenate(([0], np.cumsum(counts)[:-1]))
    pos = np.arange(n_edges) - grp_starts[key[order]]
    k = key[order]
    pair = k // (2 * GS)
    sec = k % (2 * GS)
    dest = pair * paircap + sec_starts[sec] + pos

    slot_edge = np.full(t_all // GS * paircap, -1, np.int64)
    slot_edge[dest] = order
    valid = slot_edge >= 0
    e_idx = np.where(valid, slot_edge, 0)
    src_s = src[e_idx]
    dst_s = dst[e_idx]

    # tile owning each slot (within pair: sections alternate t0/t1)
    nchunk_pair = GS * ct
    chunk_par = np.arange(nchunk_pair) % GS         # within lo block then hi
    chunk_par = np.concatenate([np.repeat(np.arange(GS), cl),
                                np.repeat(np.arange(GS), ch)])
    slot_tile = (np.repeat(np.arange(t_all // GS), paircap) * GS
                 + np.repeat(np.tile(chunk_par, t_all // GS), P))

    kvidx = np.where(valid, np.where(src_s < half, src_s, src_s - half), 0)
    kvidx = kvidx.astype(np.int16).reshape(t_all // GS, paircap)
    core_base = (slot_tile // t_core) * n_core
    qidx = np.where(valid, dst_s - core_base, 0).astype(np.int16)
    qidx = qidx.reshape(t_all // GS, paircap)
    dstrel = np.where(valid, dst_s - slot_tile * P, -1).astype(ml_dtypes.bfloat16)
    dstrel = dstrel.reshape(t_all // GS, nchunk_pair, P)
    ea_slots = np.where(valid[:, None], ea[e_idx], 0).astype(ml_dtypes.bfloat16)
    eaT = np.ascontiguousarray(
        ea_slots.reshape(t_all // GS, paircap, fe).transpose(0, 2, 1)
    )                                               # [pairs, FE, paircap] bf16

    # --- static per-chunk-index node bands (uniform across tiles/cores so
    # the SPMD program is identical): chunk k of any tile only contains
    # edges whose dstrel falls in [bs[k], bs[k]+W).
    dr = np.where(valid, dst_s - slot_tile * P, 10 ** 6)
    dr = dr.reshape(t_all // GS, nchunk_pair, P)
    lo_k = dr.min(axis=(0, 2))                      # per chunk-in-pair index
    drx = np.where(valid, dst_s - slot_tile * P, -(10 ** 6))
    hi_k = drx.reshape(t_all // GS, nchunk_pair, P).max(axis=(0, 2))
    lo_k = np.minimum(lo_k, P - 1)
    hi_k = np.maximum(hi_k, 0)
    # PE requires PSUM out base partitions in {0, 32, 64}: 32-align bands
    W = 64
    bs_k = np.clip((lo_k // 32) * 32, 0, P - W).astype(np.int64)
    if not (hi_k <= bs_k + W - 1).all():
        W = P
        bs_k = np.zeros_like(bs_k)
    # shift dstrel into band-relative coords (invalid slots go negative)
    dstrel = (dstrel.astype(np.float32)
              - bs_k[None, :, None]).astype(ml_dtypes.bfloat16)

    kvw = _wrap_idx(kvidx)                          # [pairs, 128, paircap//16]
    qw = _wrap_idx(qidx)
    dstrel_t = np.ascontiguousarray(dstrel.transpose(0, 2, 1))  # [pairs,128,nck]

    x_pad = np.zeros((n_pad, x.shape[1]), np.float32)
    x_pad[:n_nodes] = x
    xT_full = np.ascontiguousarray(x_pad.T.astype(ml_dtypes.bfloat16))

    p_core = t_core // GS
    percore = []
    for c in range(NCORES):
        ps = slice(c * p_core, (c + 1) * p_core)
        percore.append(
            dict(
                xT=xT_full,
                xT_own=np.ascontiguousarray(
                    xT_full[:, c * n_core:(c + 1) * n_core]),
                eaT=np.ascontiguousarray(eaT[ps]),
                kvidx=np.ascontiguousarray(
                    kvw[ps].transpose(1, 0, 2).reshape(P, -1)),
                qidx=np.ascontiguousarray(
                    qw[ps].transpose(1, 0, 2).reshape(P, -1)),
                dstrel=np.ascontiguousarray(
                    dstrel_t[ps].transpose(1, 0, 2).reshape(P, -1)),
            )
        )
    dcfg = dict(
        t_core=t_core, cl=cl, ch=ch, half=half, n_pad=n_pad, n_core=n_core,
        fn=x.shape[1], fe=fe, h=HEADS, c1=C1, c2=C2, ncls=N_CLASSES,
        W=W, bands=tuple(int(b) for b in bs_k),
    )
    return percore, dcfg


# ----------------------------------------------------------------------------
# device program
# ----------------------------------------------------------------------------

def _edge_layer(nc, tc, pool, psum, cfg, consts, layer):
    """One TransformerConv edge pass over this core's tiles, by pairs.

    Gathers per-edge Q (by dst) and K|V (by src) bf16 rows, computes the
    edge softmax without max-subtraction, and accumulates one-hot
    segment-sum matmuls into PSUM. The relu'd per-node result lands in
    layer["h_res"] ([128, t_core*c])."""
    t_core, cl, ch = cfg["t_core"], cfg["cl"], cfg["ch"]
    ct = cl + ch
    fe, H = cfg["fe"], cfg["h"]
    c = layer["c"]
    hc = H * c
    qe_w = layer["qe_w"]                 # gathered q row width (>= hc)
    iota = consts["iota"]                # [P, W] band iota
    zrow = consts["zrow"]                # [1, P] zeros (psum init)
    W = cfg["W"]
    bands = cfg["bands"]                 # per chunk-in-pair band start
    kvidx_sb, qidx_sb, dstrel_sb = consts["kvidx"], consts["qidx"], consts["dstrel"]
    scale = 1.0 / float(np.sqrt(c))
    half = cfg["half"]

    q_dram, kv_full = layer["q_dram"], layer["kv_full"]
    We_sb = layer["We_sb"]
    h_res = layer["h_res"]
    skip_all = layer["skip_all"]
    G = 6                                # chunks per DVE slab group
    TG = 8                               # tiles per epilogue batch
    npair = t_core // GS
    nck = GS * ct                        # chunks per pair
    # chunk index within pair -> tile parity
    ck_map = []
    for j in range(cl * GS):
        ck_map.append((j // cl, j % cl))
    for j in range(ch * GS):
        ck_map.append((j // ch, cl + j % ch))

    agg_grp = None
    for pr in range(npair):
        eaT_t = pool.tile([fe, nck * P], BF, tag="eaT")
        nc.sync.dma_start(out=eaT_t[:], in_=layer["eaT_dram"][pr])

        q_e = pool.tile([P, nck, qe_w], BF, tag="q_e")
        kv_e = pool.tile([P, nck, 2 * hc], BF, tag="kv_e")
        qi = qidx_sb[:, pr * nck * 8:(pr + 1) * nck * 8]
        ki = kvidx_sb[:, pr * nck * 8:(pr + 1) * nck * 8]

        # gathers: q spans the whole pair; kv split into lo/hi sections.
        # dma_gather tops out at 1024 indices per call.
        MAXC = 8

        def emit_gathers(out_tile, table_ap, idx_ap, c0, nch, elem, queue=0):
            for s0 in range(0, nch, MAXC):
                n = min(MAXC, nch - s0)
                nc.gpsimd.dma_gather(
                    out_tile[:, c0 + s0:c0 + s0 + n, :], table_ap,
                    idx_ap[:, (c0 + s0) * 8:(c0 + s0 + n) * 8],
                    n * P, n * P, elem, queue_num=queue)

        emit_gathers(q_e, q_dram[:, :], qi, 0, nck, qe_w, queue=1)
        emit_gathers(kv_e, kv_full[:half, :], ki, 0, GS * cl, 2 * hc, queue=0)
        emit_gathers(kv_e, kv_full[half:, :], ki, GS * cl, GS * ch, 2 * hc,
                     queue=2)

        # per-tile PSUM accumulators for this pair, zero-initialized by a
        # K=1 outer-product matmul (banded scatters only touch stripes)
        aggs = [psum.tile([P, H * (c + 1)], FP, space="PSUM", tag=f"agg{i}",
                          name=f"agg{i}")
                for i in range(GS)]
        for i in range(GS):
            nc.tensor.matmul(out=aggs[i][:], lhsT=zrow[:1, 0:P],
                             rhs=zrow[:1, 0:H * (c + 1)],
                             start=True, stop=False)
        nleft = [ct] * GS

        for g0 in range(0, nck, G):
            gn = min(G, nck - g0)
            e_ps = psum.tile([P, G * hc], FP, space="PSUM", tag="e_ps")
            for j in range(gn):
                nc.tensor.matmul(
                    out=e_ps[:, j * hc:(j + 1) * hc],
                    lhsT=eaT_t[:, (g0 + j) * P:(g0 + j + 1) * P],
                    rhs=We_sb[:],
                    start=True, stop=True,
                )
            # e: PSUM -> SBUF bf16 on the Activation engine
            e_sb = pool.tile([P, G * hc], BF, tag="e_sb")
            nc.scalar.activation(
                out=e_sb[:, 0:gn * hc], in_=e_ps[:, 0:gn * hc],
                func=mybir.ActivationFunctionType.Copy)
            # fused K|V += e  (all-SBUF bf16 -> 4x DVE)
            kev = pool.tile([P, G * 2 * hc], BF, tag="kev")
            nc.vector.tensor_tensor(
                out=kev[:].rearrange("p (g two f) -> p g two f", g=G,
                                     two=2)[:, 0:gn],
                in0=kv_e[:, g0:g0 + gn, :].rearrange(
                    "p g (two f) -> p g two f", two=2),
                in1=e_sb[:].rearrange("p (g f) -> p g f", g=G)[:, 0:gn, None, :]
                    .to_broadcast([P, gn, 2, hc]),
                op=mybir.AluOpType.add)
            kev4 = kev[:].rearrange("p (g two f) -> p g two f", g=G, two=2)
            # qk product (all-SBUF bf16)
            qk = pool.tile([P, G * hc], mybir.dt.float16, tag="qk")
            nc.vector.tensor_tensor(
                out=qk[:].rearrange("p (g f) -> p g f", g=G)[:, 0:gn],
                in0=q_e[:, g0:g0 + gn, 0:hc],
                in1=kev4[:, 0:gn, 0, :],
                op=mybir.AluOpType.mult)
            lg = pool.tile([P, G * H], FP, tag="lg")
            nc.vector.reduce_sum(
                out=lg[:].rearrange("p (g h) -> p g h", g=G)[:, 0:gn],
                in_=qk[:].rearrange("p (g h w) -> p g h w", g=G,
                                    h=H)[:, 0:gn],
                axis=mybir.AxisListType.X)
            p_t = pool.tile([P, G * H], BF, tag="p_t")
            nc.scalar.activation(
                out=p_t[:, 0:gn * H], in_=lg[:, 0:gn * H],
                func=mybir.ActivationFunctionType.Exp, scale=scale)
            pv = pool.tile([P, G * H * (c + 1)], BF, tag="pv")
            pv4 = pv[:].rearrange("p (g h w) -> p g h w", g=G, h=H)
            p3 = p_t[:].rearrange("p (g h) -> p g h", g=G)
            nc.vector.tensor_tensor(
                out=pv4[:, 0:gn, :, 0:c],
                in0=kev4[:, 0:gn, 1, :].rearrange("p g (h w) -> p g h w", h=H),
                in1=p3[:, 0:gn, :, None].to_broadcast([P, gn, H, c]),
                op=mybir.AluOpType.mult)
            nc.vector.tensor_copy(out=pv4[:, 0:gn, :, c], in_=p3[:, 0:gn, :])
            oh = pool.tile([P, G * W], BF, tag="oh")
            nc.vector.tensor_tensor(
                out=oh[:].rearrange("p (g f) -> p g f", g=G)[:, 0:gn],
                in0=iota[:, None, :].to_broadcast([P, gn, W]),
                in1=dstrel_sb[:, pr * nck + g0: pr * nck + g0 + gn][:, :, None]
                    .to_broadcast([P, gn, W]),
                op=mybir.AluOpType.is_equal)
            for j in range(gn):
                par, _ = ck_map[g0 + j]
                b0 = bands[g0 + j]
                nleft[par] -= 1
                rhs_pv = pv[:, j * H * (c + 1):(j + 1) * H * (c + 1)]
                if b0 == 32:
                    # PE stripe rule: base 32 allows only 32 partitions
                    nc.tensor.matmul(
                        out=aggs[par][32:64, :],
                        lhsT=oh[:, j * W:j * W + 32],
                        rhs=rhs_pv, start=False, stop=False)
                    nc.tensor.matmul(
                        out=aggs[par][64:96, :],
                        lhsT=oh[:, j * W + 32:(j + 1) * W],
                        rhs=rhs_pv, start=False, stop=nleft[par] == 0)
                else:
                    nc.tensor.matmul(
                        out=aggs[par][b0:b0 + W, :],
                        lhsT=oh[:, j * W:(j + 1) * W],
                        rhs=rhs_pv, start=False, stop=nleft[par] == 0,
                    )

        # stash the pair's PSUM aggregates; epilogues batched per TG tiles
        for i in range(GS):
            t = pr * GS + i
            tg = t % TG
            if tg == 0:
                agg_grp = pool.tile([P, TG * H * (c + 1)], FP, tag="agg_grp")
            nc.vector.tensor_copy(
                out=agg_grp[:, tg * H * (c + 1):(tg + 1) * H * (c + 1)],
                in_=aggs[i][:])
            if tg == TG - 1 or t == t_core - 1:
                n = tg + 1
                t0 = t - tg
                a4 = agg_grp[:].rearrange("p (t h w) -> p t h w", t=TG, h=H)
                rs = pool.tile([P, TG * H], FP, tag="rs")
                nc.vector.tensor_scalar(
                    out=rs[:, 0:n * H],
                    in0=a4[:, 0:n, :, c].rearrange("p t h -> p (t h)"),
                    scalar1=1e-30, scalar2=None, op0=mybir.AluOpType.add)
                nc.vector.reciprocal(out=rs[:, 0:n * H], in_=rs[:, 0:n * H])
                nc.vector.tensor_scalar(
                    out=rs[:, 0:n * H], in0=rs[:, 0:n * H], scalar1=1.0 / H,
                    scalar2=None, op0=mybir.AluOpType.mult)
                nc.vector.tensor_tensor(
                    out=a4[:, 0:n, :, 0:c], in0=a4[:, 0:n, :, 0:c],
                    in1=rs[:].rearrange("p (t h) -> p t h", t=TG)[:, 0:n, :,
                                                                  None]
                        .to_broadcast([P, n, H, c]),
                    op=mybir.AluOpType.mult)
                hsum = pool.tile([P, TG * c], FP, tag="hsum")
                nc.vector.reduce_sum(
                    out=hsum[:].rearrange("p (t w) -> p t w", t=TG)[:, 0:n],
                    in_=agg_grp[:].rearrange("p (t h w) -> p t w h", t=TG,
                                             h=H)[:, 0:n, 0:c, :],
                    axis=mybir.AxisListType.X)
                # 1/H already folded into rs; skip holds bs + mean_h bv
                nc.vector.tensor_tensor(
                    out=hsum[:, 0:n * c], in0=hsum[:, 0:n * c],
                    in1=skip_all[:, t0 * c:(t0 + n) * c],
                    op=mybir.AluOpType.add)
                nc.scalar.activation(
                    out=h_res[:, t0 * c:(t0 + n) * c], in_=hsum[:, 0:n * c],
                    func=mybir.ActivationFunctionType.Relu)


def build_device(dcfg):
    t_core, cl, ch = dcfg["t_core"], dcfg["cl"], dcfg["ch"]
    ct = cl + ch
    n_pad, n_core = dcfg["n_pad"], dcfg["n_core"]
    fn, fe, H = dcfg["fn"], dcfg["fe"], dcfg["h"]
    c1, c2, ncls = dcfg["c1"], dcfg["c2"], dcfg["ncls"]
    hc1, hc2 = H * c1, H * c2
    hid = 2 * c2
    t_all = n_pad // P
    reps = int(os.environ.get("KBUILD_REPS", "1"))

    nc = bacc.Bacc("TRN2", target_bir_lowering=False, debug=False,
                   num_devices=NCORES, num_swdge_queues=4)

    def param(name, shape, dtype=FP, out=False):
        return nc.declare_dram_parameter(name, list(shape), dtype, isOutput=out)

    xT_d = param("xT", [fn, n_pad], BF)
    xT_own_d = param("xT_own", [fn, n_core], BF)
    eaT_d = param("eaT", [t_core // GS, fe, GS * ct * P], BF)
    kvidx_d = param("kvidx", [P, t_core * ct * 8], mybir.dt.int16)
    qidx_d = param("qidx", [P, t_core * ct * 8], mybir.dt.int16)
    dstrel_d = param("dstrel", [P, t_core * ct], BF)
    wkv1_d = param("wkv1", [fn, 2 * hc1], BF)
    wqs1_d = param("wqs1", [fn, hc1 + c1], BF)
    bqs1_d = param("bqs1", [1, hc1 + c1], BF)
    we1_d = param("we1", [fe, hc1], BF)
    wkv2_d = param("wkv2", [c1, 2 * hc2], BF)
    wqs2_d = param("wqs2", [c1, hc2 + c2], BF)
    bqs2_d = param("bqs2", [1, hc2 + c2], BF)
    we2_d = param("we2", [fe, hc2], BF)
    w3_d = param("w3", [c2, hid], BF)
    b3_d = param("b3", [hid, 1], FP)
    w4_d = param("w4", [hid, ncls], BF)
    b4_d = param("b4", [ncls, 1], FP)
    out_d = param("out", [ncls, n_core], out=True)

    with tile.TileContext(nc) as tc:
        with (
            tc.tile_pool(name="res", bufs=1) as res,
            tc.tile_pool(name="dram", bufs=1, space="DRAM") as dram,
        ):
            # ---- constants / resident tensors
            ident = res.tile([P, P], FP)
            make_identity(nc, ident[:])
            W = dcfg["W"]
            iota = res.tile([P, W], BF)
            nc.gpsimd.iota(iota[:], pattern=[[1, W]], base=0,
                           channel_multiplier=0,
                           allow_small_or_imprecise_dtypes=True)
            zrow = res.tile([1, 192], BF)
            nc.vector.memset(zrow[:], 0.0)
            kvidx_sb = res.tile([P, t_core * ct * 8], mybir.dt.int16)
            nc.sync.dma_start(out=kvidx_sb[:], in_=kvidx_d[:])
            qidx_sb = res.tile([P, t_core * ct * 8], mybir.dt.int16)
            nc.sync.dma_start(out=qidx_sb[:], in_=qidx_d[:])
            dstrel_sb = res.tile([P, t_core * ct], BF)
            nc.sync.dma_start(out=dstrel_sb[:], in_=dstrel_d[:])

            def load_w(d, shape, tag, dt=BF):
                t = res.tile(list(shape), dt, tag=tag)
                nc.sync.dma_start(out=t[:], in_=d[:])
                return t

            wkv1 = load_w(wkv1_d, [fn, 2 * hc1], "wkv1")
            wqs1 = load_w(wqs1_d, [fn, hc1 + c1], "wqs1")
            bqs1 = load_w(bqs1_d, [1, hc1 + c1], "bqs1")
            we1 = load_w(we1_d, [fe, hc1], "we1")
            wkv2 = load_w(wkv2_d, [c1, 2 * hc2], "wkv2")
            wqs2 = load_w(wqs2_d, [c1, hc2 + c2], "wqs2")
            bqs2 = load_w(bqs2_d, [1, hc2 + c2], "bqs2")
            we2 = load_w(we2_d, [fe, hc2], "we2")
            w3 = load_w(w3_d, [c2, hid], "w3")
            b3 = load_w(b3_d, [hid, 1], "b3", FP)
            w4 = load_w(w4_d, [hid, ncls], "w4")
            b4 = load_w(b4_d, [ncls, 1], "b4", FP)
            ones_row = res.tile([1, P], BF)
            nc.vector.memset(ones_row[:], 1.0)

            skip1_all = res.tile([P, t_core * c1], FP)
            skip2_all = res.tile([P, t_core * c2], FP)
            h1_res = res.tile([P, t_core * c1], FP)
            h1T_bf = res.tile([c1, t_core * P], BF)
            h2_res = res.tile([P, t_core * c2], FP)
            h2T_res = res.tile([c2, t_core * P], BF)

            # ---- internal DRAM
            kv1_full = dram.tile([n_pad, 2 * hc1], BF)
            q1_dram = dram.tile([n_core, hc1], BF)
            h1T_shard = dram.tile([t_core, c1 * P], BF)
            h1T_full = dram.tile([t_all, c1 * P], BF)
            kv2_full = dram.tile([n_pad, 2 * hc2], BF)
            q2_dram = dram.tile([n_core, 2 * hc2], BF)   # q2 padded to 256B

            def emit_pipeline():
                # ---- phase A: full kv1 projection (all tiles, redundant per
                # core) + own q1/skip1
                XB = 8                                   # xT tiles per DMA
                with (tc.tile_pool(name="psumA", bufs=2, space="PSUM") as psum,
                      tc.tile_pool(name="poolA", bufs=2) as pool):
                  for tb in range(0, t_all, XB):
                      xT_blk = pool.tile([fn, XB * P], BF, tag="xT_blk")
                      nc.sync.dma_start(
                          out=xT_blk[:], in_=xT_d[:, tb * P:(tb + XB) * P])
                      kv_stage = pool.tile([P, XB * 2 * hc1], BF, tag="kv_st")
                      for i in range(XB):
                          pr_ps = psum.tile([P, 2 * hc1], FP, space="PSUM",
                                            tag="pr_ps")
                          nc.tensor.matmul(out=pr_ps[:],
                                           lhsT=xT_blk[:, i * P:(i + 1) * P],
                                           rhs=wkv1[:], start=True, stop=True)
                          if i % 2 == 0:
                              nc.scalar.activation(
                                  out=kv_stage[:, i * 2 * hc1:(i + 1) * 2 * hc1],
                                  in_=pr_ps[:],
                                  func=mybir.ActivationFunctionType.Copy)
                          else:
                              nc.vector.tensor_copy(
                                  out=kv_stage[:, i * 2 * hc1:(i + 1) * 2 * hc1],
                                  in_=pr_ps[:])
                      nc.sync.dma_start(
                          out=kv1_full[tb * P:(tb + XB) * P, :].rearrange(
                              "(t p) w -> p t w", p=P),
                          in_=kv_stage[:].rearrange("p (t w) -> p t w", t=XB))

                  # own q1/skip1 projections (bias row folded: [bq1 | bs1'])
                  QB = 8
                  for tb in range(0, t_core, QB):
                      bn = min(QB, t_core - tb)
                      xT_blk = pool.tile([fn, QB * P], BF, tag="xTq_blk")
                      nc.sync.dma_start(
                          out=xT_blk[:, 0:bn * P],
                          in_=xT_own_d[:, tb * P:(tb + bn) * P])
                      q_stage = pool.tile([P, QB * hc1], BF, tag="q_st")
                      for i in range(bn):
                          t = tb + i
                          qs_ps = psum.tile([P, hc1 + c1], FP, space="PSUM",
                                            tag="qs_ps")
                          nc.tensor.matmul(out=qs_ps[:],
                                           lhsT=xT_blk[:, i * P:(i + 1) * P],
                                           rhs=wqs1[:], start=True, stop=False)
                          nc.tensor.matmul(out=qs_ps[:], lhsT=ones_row[:1, :],
                                           rhs=bqs1[:1, :], start=False,
                                           stop=True)
                          nc.scalar.activation(
                              out=q_stage[:, i * hc1:(i + 1) * hc1],
                              in_=qs_ps[:, 0:hc1],
                              func=mybir.ActivationFunctionType.Copy)
                          nc.vector.tensor_copy(
                              out=skip1_all[:, t * c1:(t + 1) * c1],
                              in_=qs_ps[:, hc1:])
                      nc.sync.dma_start(
                          out=q1_dram[tb * P:(tb + bn) * P, :].rearrange(
                              "(t p) w -> p t w", p=P),
                          in_=q_stage[:, 0:bn * hc1].rearrange(
                              "p (t w) -> p t w", t=bn))

                consts = dict(iota=iota, zrow=zrow, kvidx=kvidx_sb,
                              qidx=qidx_sb, dstrel=dstrel_sb)

                # ---- phase B: layer-1 edge pass
                with (tc.tile_pool(name="psumB", bufs=2, space="PSUM") as psum,
                      tc.tile_pool(name="poolB", bufs=2) as pool):
                    _edge_layer(nc, tc, pool, psum, dcfg, consts, dict(
                        c=c1, q_dram=q1_dram, qe_w=hc1, kv_full=kv1_full,
                        We_sb=we1, eaT_dram=eaT_d, skip_all=skip1_all[:],
                        h_res=h1_res[:]))

                # ---- phase C: transpose own h1 -> bf16, ship to allgather
                with tc.tile_pool(name="psumC", bufs=2, space="PSUM") as psum:
                    for t in range(t_core):
                        h1T_ps = psum.tile([c1, P], FP, space="PSUM",
                                           tag="h1T_ps")
                        nc.tensor.transpose(
                            out=h1T_ps[:], in_=h1_res[:, t * c1:(t + 1) * c1],
                            identity=ident[:])
                        eng = nc.scalar if t % 2 == 0 else nc.vector
                        if t % 2 == 0:
                            nc.scalar.activation(
                                out=h1T_bf[:, t * P:(t + 1) * P],
                                in_=h1T_ps[:],
                                func=mybir.ActivationFunctionType.Copy)
                        else:
                            nc.vector.tensor_copy(
                                out=h1T_bf[:, t * P:(t + 1) * P],
                                in_=h1T_ps[:])
                    nc.sync.dma_start(
                        out=h1T_shard[:].rearrange("t (c p) -> c t p", c=c1),
                        in_=h1T_bf[:].rearrange("c (t p) -> c t p", t=t_core))

                nc.gpsimd.collective_compute(
                    "AllGather", mybir.AluOpType.bypass,
                    replica_groups=[list(range(NCORES))],
                    ins=[h1T_shard[:].opt()], outs=[h1T_full[:].opt()])

                # ---- phase D: full kv2 projection + own q2/skip2
                HB = 8
                with (tc.tile_pool(name="psumD", bufs=2, space="PSUM") as psum,
                      tc.tile_pool(name="poolD", bufs=2) as pool):
                  for tb in range(0, t_all, HB):
                      hT_blk = pool.tile([c1, HB * P], BF, tag="hT_blk")
                      nc.sync.dma_start(
                          out=hT_blk[:].rearrange("c (t p) -> c t p", t=HB),
                          in_=h1T_full[tb:tb + HB].rearrange(
                              "t (c p) -> c t p", c=c1))
                      kv_stage = pool.tile([P, HB * 2 * hc2], BF, tag="kv2_st")
                      for i in range(HB):
                          pr_ps = psum.tile([P, 2 * hc2], FP, space="PSUM",
                                            tag="p2_ps")
                          nc.tensor.matmul(out=pr_ps[:],
                                           lhsT=hT_blk[:, i * P:(i + 1) * P],
                                           rhs=wkv2[:], start=True, stop=True)
                          if i % 2 == 0:
                              nc.scalar.activation(
                                  out=kv_stage[:, i * 2 * hc2:(i + 1) * 2 * hc2],
                                  in_=pr_ps[:],
                                  func=mybir.ActivationFunctionType.Copy)
                          else:
                              nc.vector.tensor_copy(
                                  out=kv_stage[:, i * 2 * hc2:(i + 1) * 2 * hc2],
                                  in_=pr_ps[:])
                      nc.sync.dma_start(
                          out=kv2_full[tb * P:(tb + HB) * P, :].rearrange(
                              "(t p) w -> p t w", p=P),
                          in_=kv_stage[:].rearrange("p (t w) -> p t w", t=HB))

                  QB = 8
                  for tb in range(0, t_core, QB):
                      bn = min(QB, t_core - tb)
                      q_stage = pool.tile([P, QB * 2 * hc2], BF, tag="q2_st")
                      if tb < 2 * QB:
                          nc.vector.memset(
                              q_stage[:].rearrange(
                                  "p (t w) -> p t w", t=QB)[:, :, hc2:],
                              0.0)
                      for i in range(bn):
                          t = tb + i
                          qs_ps = psum.tile([P, hc2 + c2], FP, space="PSUM",
                                            tag="q2s_ps")
                          nc.tensor.matmul(
                              out=qs_ps[:],
                              lhsT=h1T_bf[:, t * P:(t + 1) * P],
                              rhs=wqs2[:], start=True, stop=False)
                          nc.tensor.matmul(out=qs_ps[:], lhsT=ones_row[:1, :],
                                           rhs=bqs2[:1, :], start=False,
                                           stop=True)
                          nc.scalar.activation(
                              out=q_stage[:, i * 2 * hc2:i * 2 * hc2 + hc2],
                              in_=qs_ps[:, 0:hc2],
                              func=mybir.ActivationFunctionType.Copy)
                          nc.vector.tensor_copy(
                              out=skip2_all[:, t * c2:(t + 1) * c2],
                              in_=qs_ps[:, hc2:])
                      nc.sync.dma_start(
                          out=q2_dram[tb * P:(tb + bn) * P, :].rearrange(
                              "(t p) w -> p t w", p=P),
                          in_=q_stage[:, 0:bn * 2 * hc2].rearrange(
                              "p (t w) -> p t w", t=bn))

                # ---- phase E: layer-2 edge pass
                with (tc.tile_pool(name="psumE", bufs=2, space="PSUM") as psum,
                      tc.tile_pool(name="poolE", bufs=2) as pool):
                    _edge_layer(nc, tc, pool, psum, dcfg, consts, dict(
                        c=c2, q_dram=q2_dram, qe_w=2 * hc2, kv_full=kv2_full,
                        We_sb=we2, eaT_dram=eaT_d, skip_all=skip2_all[:],
                        h_res=h2_res[:]))
                    for t in range(t_core):
                        h2T_ps = psum.tile([c2, P], FP, space="PSUM",
                                           tag="h2T_ps")
                        nc.tensor.transpose(
                            out=h2T_ps[:], in_=h2_res[:, t * c2:(t + 1) * c2],
                            identity=ident[:])
                        nc.vector.tensor_copy(
                            out=h2T_res[:, t * P:(t + 1) * P], in_=h2T_ps[:])

                # ---- phase F: dense head (output transposed [ncls, n_core])
                CHUNK = 512
                with (tc.tile_pool(name="psumF", bufs=2, space="PSUM") as psum,
                      tc.tile_pool(name="poolF", bufs=2) as pool):
                  for k0 in range(0, n_core, CHUNK):
                      kn = min(CHUNK, n_core - k0)
                      h3_ps = psum.tile([hid, CHUNK], FP, space="PSUM",
                                        tag="h3_ps")
                      nc.tensor.matmul(out=h3_ps[:, 0:kn], lhsT=w3[:],
                                       rhs=h2T_res[:, k0:k0 + kn], start=True,
                                       stop=True)
                      h3_sb = pool.tile([hid, CHUNK], BF, tag="h3_sb")
                      nc.scalar.activation(
                          out=h3_sb[:, 0:kn], in_=h3_ps[:, 0:kn],
                          func=mybir.ActivationFunctionType.Relu,
                          bias=b3[:, 0:1])
                      o_ps = psum.tile([ncls, CHUNK], FP, space="PSUM",
                                       tag="o_ps")
                      nc.tensor.matmul(out=o_ps[:, 0:kn], lhsT=w4[:],
                                       rhs=h3_sb[:, 0:kn], start=True,
                                       stop=True)
                      o_sb = pool.tile([ncls, CHUNK], FP, tag="o_sb")
                      nc.vector.tensor_scalar(
                          out=o_sb[:, 0:kn], in0=o_ps[:, 0:kn],
                          scalar1=b4[:, 0:1],
                          scalar2=None, op0=mybir.AluOpType.add)
                      nc.sync.dma_start(out=out_d[:, k0:k0 + kn],
                                        in_=o_sb[:, 0:kn])

            for _rep in range(reps):
                emit_pipeline()

    nc.compile()
    return nc


# ----------------------------------------------------------------------------
# entry point
# ----------------------------------------------------------------------------

_CACHE = {}


def _get_nc(dcfg):
    key = (tuple(sorted(dcfg.items())), os.environ.get("KBUILD_REPS", "1"))
    if key not in _CACHE:
        _CACHE[key] = build_device(dcfg)
    return _CACHE[key]


def prepare_in_maps(inputs):
    x = np.asarray(inputs["x"], np.float32)
    n_nodes = x.shape[0]
    n_edges = np.asarray(inputs["edge_index"]).shape[1]
    percore, dcfg = host_prep(
        x, np.asarray(inputs["edge_index"]),
        np.asarray(inputs["edge_attr"], np.float32),
        n_nodes, n_edges, np.asarray(inputs["edge_attr"]).shape[1])
    bf = lambda a: np.ascontiguousarray(
        np.asarray(a, np.float32).astype(ml_dtypes.bfloat16))
    f32 = lambda a: np.ascontiguousarray(np.asarray(a, np.float32))
    i = inputs
    H = HEADS
    # fold mean-over-heads of bv into the skip bias (sum_e alpha = 1)
    bs1f = (np.asarray(i["bs1"], np.float32)
            + np.asarray(i["bv1"], np.float32).reshape(H, C1).mean(0))
    bs2f = (np.asarray(i["bs2"], np.float32)
            + np.asarray(i["bv2"], np.float32).reshape(H, C2).mean(0))
    weights = dict(
        wkv1=bf(np.concatenate([f32(i["Wk1"]), f32(i["Wv1"])], axis=1)),
        wqs1=bf(np.concatenate([f32(i["Wq1"]), f32(i["Ws1"])], axis=1)),
        bqs1=bf(np.concatenate([f32(i["bq1"]), bs1f])[None, :]),
        we1=bf(i["We1"]),
        wkv2=bf(np.concatenate([f32(i["Wk2"]), f32(i["Wv2"])], axis=1)),
        wqs2=bf(np.concatenate([f32(i["Wq2"]), f32(i["Ws2"])], axis=1)),
        bqs2=bf(np.concatenate([f32(i["bq2"]), bs2f])[None, :]),
        we2=bf(i["We2"]),
        w3=bf(i["W3"]), b3=f32(i["b3"])[:, None],
        w4=bf(i["W4"]), b4=f32(i["b4"])[:, None],
    )
    return [dict(pc, **weights) for pc in percore], dcfg


def assemble_output(res, inputs):
    n_nodes = np.asarray(inputs["x"]).shape[0]
    out = np.concatenate([res.results[i]["out"].T for i in range(NCORES)])
    return np.ascontiguousarray(out[:n_nodes])


def kernel(x, edge_index, edge_attr,
           Wq1, bq1, Wk1, bk1, Wv1, bv1, We1, Ws1, bs1,
           Wq2, bq2, Wk2, bk2, Wv2, bv2, We2, Ws2, bs2,
           W3, b3, W4, b4):
    inputs = dict(
        x=x, edge_index=edge_index, edge_attr=edge_attr,
        Wq1=Wq1, bq1=bq1, Wk1=Wk1, bk1=bk1, Wv1=Wv1, bv1=bv1, We1=We1,
        Ws1=Ws1, bs1=bs1,
        Wq2=Wq2, bq2=bq2, Wk2=Wk2, bk2=bk2, Wv2=Wv2, bv2=bv2, We2=We2,
        Ws2=Ws2, bs2=bs2,
        W3=W3, b3=b3, W4=W4, b4=b4,
    )
    in_maps, dcfg = prepare_in_maps(inputs)
    nc = _get_nc(dcfg)
    res = run_bass_kernel_spmd(nc, in_maps, core_ids=list(range(NCORES)))
    return assemble_output(res, inputs)


# revision 41
# speedup vs baseline: 28.4769x; 2.7342x over previous
"""Trainium2 Bass kernel for a 2-layer TransformerConv GNN + MLP head.

Contract: kernel(**inputs) takes the FULL inputs (as produced by
setup_inputs()) and returns the FULL [N, 2] output, running the compute
on 8 NeuronCores via run_bass_kernel_spmd.

Sharding: nodes are padded to 50176 = 8 * 49 * 128 and split into 8
contiguous ranges of 49 node-tiles (128 nodes each). Each core owns the
edges whose *target* (dst) falls in its range (edge/data parallel with
disjoint segment sums -> no all-reduce needed).

v2 design notes:
- No kv1 AllGather: x is replicated (bf16, transposed); every core
  computes the FULL kv1 projection table redundantly (PE is idle) and
  writes it to local DRAM. The only collective is a small AllGather of
  per-tile-transposed h1 (bf16, 3.2MB total) between the layers.
- Gather tables are bf16 (kv rows 512B / 256B) to halve gather DMA.
- Bias algebra: bk is dropped (adding q.bk to all logits of one dst is
  softmax-invariant -> exactly no effect on alpha); bv is folded into
  the skip bias on the host (sum_e alpha = 1); bq/bs ride the q/skip
  projection of the 49 own tiles only.
- Edge pipeline per 128-node tile: per-edge Q (by dst) and K|V (by src)
  rows fetched with dma_gather (int16 indices; kv table split in two
  halves for the int16 range; gather calls span PAIRS of tiles to halve
  the 994ns/call SWDGE prep); e = ea @ We on the PE from bf16
  transposed edge attributes; softmax without max-subtraction
  (mathematically identical here; exp cannot overflow fp32 at these
  magnitudes); segment sums over edges are one-hot matmuls into PSUM.
- DVE hot path is all-SBUF bf16 (4x/2x DVE perf modes); the PSUM->SBUF
  copy of e runs on the Activation engine.
"""

import sys

sys.path.insert(0, "/opt/trn_rl_repo")

import os

import numpy as np
import ml_dtypes

import concourse.bacc as bacc
import concourse.bass as bass
import concourse.mybir as mybir
import concourse.tile as tile
from concourse.bass_utils import run_bass_kernel_spmd
from concourse.masks import make_identity

P = 128
NCORES = 8
FP = mybir.dt.float32
BF = mybir.dt.bfloat16
GS = 2                      # tiles per gather-call group

# problem dims (hardcoded per contract)
N_NODES = 50000
N_EDGES = 800000
F_NODE = 128
F_EDGE = 32
HEADS = 4
C1 = 32
C2 = 16
N_CLASSES = 2


# ----------------------------------------------------------------------------
# host-side preprocessing
# ----------------------------------------------------------------------------

def _wrap_idx(a):
    """[T, S] int16 -> dma_gather wrapped layout [T, 128, S//16]:
    index i of a call lands at [i % 16, i // 16], replicated x8 down
    the partitions (each GPSIMD core reads its own 16-partition group)."""
    T, S = a.shape
    w = np.ascontiguousarray(a.reshape(T, S // 16, 16).transpose(0, 2, 1))
    return np.tile(w, (1, 8, 1))


def host_prep(x, edge_index, edge_attr, n_nodes, n_edges, fe):
    """Build per-core device inputs for the edge phase.

    Slot layout per PAIR of consecutive tiles (t0=2p, t1=2p+1):
      [lo(t0) cl chunks | lo(t1) cl | hi(t0) ch | hi(t1) ch]
    so one gather call covers the lo (resp. hi) sections of both tiles.
    """
    t_total = -(-n_nodes // P)                      # ceil
    t_core = -(-t_total // NCORES)
    if t_core % GS:
        t_core += GS - t_core % GS
    t_all = t_core * NCORES
    n_pad = t_all * P
    n_core = t_core * P
    half = (n_pad // 2 + P - 1) // P * P            # split point for int16 tables
    assert half < 32768 and n_pad - half < 32768 and n_core < 32768

    src = np.asarray(edge_index[0], dtype=np.int64)
    dst = np.asarray(edge_index[1], dtype=np.int64)
    ea = np.asarray(edge_attr, dtype=np.float32)

    tile_of = dst // P
    hi = (src >= half).astype(np.int64)
    # group = pair index; section within pair = hi*2 + (tile parity)
    key = (tile_of // GS) * 2 * GS + hi * GS + (tile_of % GS)
    order = np.lexsort((dst, key))      # dst-sorted within each section
    counts = np.bincount(key, minlength=t_all * 2)
    # per (tile, half) chunk capacity, uniform across all tiles
    cnt2 = counts.reshape(t_all // GS, 2, GS)
    cl = int(-(-cnt2[:, 0, :].max() // P))          # lo chunks per tile
    ch = int(-(-cnt2[:, 1, :].max() // P))          # hi chunks per tile
    ct = cl + ch
    cap = ct * P                                    # slots per tile
    paircap = GS * cap

    # destination slot of each edge, in pair-call order
    # key % (2*GS): hi*GS + parity ; order lo(t0),lo(t1),hi(t0),hi(t1)
    sec_sizes = np.array([cl * P] * GS + [ch * P] * GS)
    sec_starts = np.concatenate(([0], np.cumsum(sec_sizes)[:-1]))
    grp_starts = np.concatenate(([0], np.cumsum(counts)[:-1]))
    pos = np.arange(n_edges) - grp_starts[key[order]]
    k = key[order]
    pair = k // (2 * GS)
    sec = k % (2 * GS)
    dest = pair * paircap + sec_starts[sec] + pos

    slot_edge = np.full(t_all // GS * paircap, -1, np.int64)
    slot_edge[dest] = order
    valid = slot_edge >= 0
    e_idx = np.where(valid, slot_edge, 0)
    src_s = src[e_idx]
    dst_s = dst[e_idx]

    # tile owning each slot (within pair: sections alternate t0/t1)
    nchunk_pair = GS * ct
    chunk_par = np.arange(nchunk_pair) % GS         # within lo block then hi
    chunk_par = np.concatenate([np.repeat(np.arange(GS), cl),
                                np.repeat(np.arange(GS), ch)])
    slot_tile = (np.repeat(np.arange(t_all // GS), paircap) * GS
                 + np.repeat(np.tile(chunk_par, t_all // GS), P))

    kvidx = np.where(valid, np.where(src_s < half, src_s, src_s - half), 0)
    kvidx = kvidx.astype(np.int16).reshape(t_all // GS, paircap)
    core_base = (slot_tile // t_core) * n_core
    qidx = np.where(valid, dst_s - core_base, 0).astype(np.int16)
    qidx = qidx.reshape(t_all // GS, paircap)
    dstrel = np.where(valid, dst_s - slot_tile * P, -1).astype(ml_dtypes.bfloat16)
    dstrel = dstrel.reshape(t_all // GS, nchunk_pair, P)
    ea_slots = np.where(valid[:, None], ea[e_idx], 0).astype(ml_dtypes.bfloat16)
    eaT = np.ascontiguousarray(
        ea_slots.reshape(t_all // GS, paircap, fe).transpose(0, 2, 1)
    )                                               # [pairs, FE, paircap] bf16

    # --- static per-chunk-index node bands (uniform across tiles/cores so
    # the SPMD program is identical): chunk k of any tile only contains
    # edges whose dstrel falls in [bs[k], bs[k]+W).
    dr = np.where(valid, dst_s - slot_tile * P, 10 ** 6)
    dr = dr.reshape(t_all // GS, nchunk_pair, P)
    lo_k = dr.min(axis=(0, 2))                      # per chunk-in-pair index
    drx = np.where(valid, dst_s - slot_tile * P, -(10 ** 6))
    hi_k = drx.reshape(t_all // GS, nchunk_pair, P).max(axis=(0, 2))
    lo_k = np.minimum(lo_k, P - 1)
    hi_k = np.maximum(hi_k, 0)
    # PE requires PSUM out base partitions in {0, 32, 64}: 32-align bands
    W = 64
    bs_k = np.clip((lo_k // 32) * 32, 0, P - W).astype(np.int64)
    if not (hi_k <= bs_k + W - 1).all():
        W = P
        bs_k = np.zeros_like(bs_k)
    # shift dstrel into band-relative coords (invalid slots go negative)
    dstrel = (dstrel.astype(np.float32)
              - bs_k[None, :, None]).astype(ml_dtypes.bfloat16)

    kvw = _wrap_idx(kvidx)                          # [pairs, 128, paircap//16]
    qw = _wrap_idx(qidx)
    dstrel_t = np.ascontiguousarray(dstrel.transpose(0, 2, 1))  # [pairs,128,nck]

    x_pad = np.zeros((n_pad, x.shape[1]), np.float32)
    x_pad[:n_nodes] = x
    xT_full = np.ascontiguousarray(x_pad.T.astype(ml_dtypes.bfloat16))

    p_core = t_core // GS
    percore = []
    for c in range(NCORES):
        ps = slice(c * p_core, (c + 1) * p_core)
        percore.append(
            dict(
                xT=xT_full,
                xT_own=np.ascontiguousarray(
                    xT_full[:, c * n_core:(c + 1) * n_core]),
                eaT=np.ascontiguousarray(eaT[ps]),
                kvidx=np.ascontiguousarray(
                    kvw[ps].transpose(1, 0, 2).reshape(P, -1)),
                qidx=np.ascontiguousarray(
                    qw[ps].transpose(1, 0, 2).reshape(P, -1)),
                dstrel=np.ascontiguousarray(
                    dstrel_t[ps].transpose(1, 0, 2).reshape(P, -1)),
            )
        )
    dcfg = dict(
        t_core=t_core, cl=cl, ch=ch, half=half, n_pad=n_pad, n_core=n_core,
        fn=x.shape[1], fe=fe, h=HEADS, c1=C1, c2=C2, ncls=N_CLASSES,
        W=W, bands=tuple(int(b) for b in bs_k),
    )
    return percore, dcfg


# ----------------------------------------------------------------------------
# device program
# ----------------------------------------------------------------------------

def _edge_layer(nc, tc, pool, psum, cfg, consts, layer):
    """One TransformerConv edge pass over this core's tiles, by pairs.

    Gathers per-edge Q (by dst) and K|V (by src) bf16 rows, computes the
    edge softmax without max-subtraction, and accumulates one-hot
    segment-sum matmuls into PSUM. The relu'd per-node result lands in
    layer["h_res"] ([128, t_core*c])."""
    t_core, cl, ch = cfg["t_core"], cfg["cl"], cfg["ch"]
    ct = cl + ch
    fe, H = cfg["fe"], cfg["h"]
    c = layer["c"]
    hc = H * c
    qe_w = layer["qe_w"]                 # gathered q row width (>= hc)
    iota = consts["iota"]                # [P, W] band iota
    zrow = consts["zrow"]                # [1, P] zeros (psum init)
    W = cfg["W"]
    bands = cfg["bands"]                 # per chunk-in-pair band start
    kvidx_sb, qidx_sb, dstrel_sb = consts["kvidx"], consts["qidx"], consts["dstrel"]
    scale = 1.0 / float(np.sqrt(c))
    half = cfg["half"]

    q_dram, kv_full = layer["q_dram"], layer["kv_full"]
    We_sb = layer["We_sb"]
    h_res = layer["h_res"]
    skip_all = layer["skip_all"]
    G = 6                                # chunks per DVE slab group
    TG = 8                               # tiles per epilogue batch
    npair = t_core // GS
    nck = GS * ct                        # chunks per pair
    # chunk index within pair -> tile parity
    ck_map = []
    for j in range(cl * GS):
        ck_map.append((j // cl, j % cl))
    for j in range(ch * GS):
        ck_map.append((j // ch, cl + j % ch))

    agg_grp = None
    for pr in range(npair):
        eaT_t = pool.tile([fe, nck * P], BF, tag="eaT")
        nc.sync.dma_start(out=eaT_t[:], in_=layer["eaT_dram"][pr])

        q_e = pool.tile([P, nck, qe_w], BF, tag="q_e")
        kv_e = pool.tile([P, nck, 2 * hc], BF, tag="kv_e")
        qi = qidx_sb[:, pr * nck * 8:(pr + 1) * nck * 8]
        ki = kvidx_sb[:, pr * nck * 8:(pr + 1) * nck * 8]

        # gathers: q spans the whole pair; kv split into lo/hi sections.
        # dma_gather tops out at 1024 indices per call (KB_MAXC to probe).
        MAXC = int(os.environ.get("KB_MAXC", "8"))

        def emit_gathers(out_tile, table_ap, idx_ap, c0, nch, elem, queue=0):
            for s0 in range(0, nch, MAXC):
                n = min(MAXC, nch - s0)
                nc.gpsimd.dma_gather(
                    out_tile[:, c0 + s0:c0 + s0 + n, :], table_ap,
                    idx_ap[:, (c0 + s0) * 8:(c0 + s0 + n) * 8],
                    n * P, n * P, elem, queue_num=queue)

        emit_gathers(q_e, q_dram[:, :], qi, 0, nck, qe_w, queue=1)
        emit_gathers(kv_e, kv_full[:half, :], ki, 0, GS * cl, 2 * hc, queue=0)
        emit_gathers(kv_e, kv_full[half:, :], ki, GS * cl, GS * ch, 2 * hc,
                     queue=2)

        # per-tile PSUM accumulators for this pair, zero-initialized by a
        # K=1 outer-product matmul (banded scatters only touch stripes)
        aggs = [psum.tile([P, H * (c + 1)], FP, space="PSUM", tag=f"agg{i}",
                          name=f"agg{i}")
                for i in range(GS)]
        for i in range(GS):
            nc.tensor.matmul(out=aggs[i][:], lhsT=zrow[:1, 0:P],
                             rhs=zrow[:1, 0:H * (c + 1)],
                             start=True, stop=False)
        nleft = [ct] * GS

        for g0 in range(0, nck, G):
            gn = min(G, nck - g0)
            e_ps = psum.tile([P, G * hc], FP, space="PSUM", tag="e_ps")
            for j in range(gn):
                nc.tensor.matmul(
                    out=e_ps[:, j * hc:(j + 1) * hc],
                    lhsT=eaT_t[:, (g0 + j) * P:(g0 + j + 1) * P],
                    rhs=We_sb[:],
                    start=True, stop=True,
                )
            # e: PSUM -> SBUF bf16 on the Activation engine
            e_sb = pool.tile([P, G * hc], BF, tag="e_sb")
            nc.scalar.activation(
                out=e_sb[:, 0:gn * hc], in_=e_ps[:, 0:gn * hc],
                func=mybir.ActivationFunctionType.Copy)
            # fused K|V += e  (all-SBUF bf16 -> 4x DVE)
            kev = pool.tile([P, G * 2 * hc], BF, tag="kev")
            nc.vector.tensor_tensor(
                out=kev[:].rearrange("p (g two f) -> p g two f", g=G,
                                     two=2)[:, 0:gn],
                in0=kv_e[:, g0:g0 + gn, :].rearrange(
                    "p g (two f) -> p g two f", two=2),
                in1=e_sb[:].rearrange("p (g f) -> p g f", g=G)[:, 0:gn, None, :]
                    .to_broadcast([P, gn, 2, hc]),
                op=mybir.AluOpType.add)
            kev4 = kev[:].rearrange("p (g two f) -> p g two f", g=G, two=2)
            # qk product (all-SBUF bf16)
            qk = pool.tile([P, G * hc], mybir.dt.float16, tag="qk")
            nc.vector.tensor_tensor(
                out=qk[:].rearrange("p (g f) -> p g f", g=G)[:, 0:gn],
                in0=q_e[:, g0:g0 + gn, 0:hc],
                in1=kev4[:, 0:gn, 0, :],
                op=mybir.AluOpType.mult)
            lg = pool.tile([P, G * H], FP, tag="lg")
            nc.vector.reduce_sum(
                out=lg[:].rearrange("p (g h) -> p g h", g=G)[:, 0:gn],
                in_=qk[:].rearrange("p (g h w) -> p g h w", g=G,
                                    h=H)[:, 0:gn],
                axis=mybir.AxisListType.X)
            p_t = pool.tile([P, G * H], BF, tag="p_t")
            nc.scalar.activation(
                out=p_t[:, 0:gn * H], in_=lg[:, 0:gn * H],
                func=mybir.ActivationFunctionType.Exp, scale=scale)
            pv = pool.tile([P, G * H * (c + 1)], BF, tag="pv")
            pv4 = pv[:].rearrange("p (g h w) -> p g h w", g=G, h=H)
            p3 = p_t[:].rearrange("p (g h) -> p g h", g=G)
            nc.vector.tensor_tensor(
                out=pv4[:, 0:gn, :, 0:c],
                in0=kev4[:, 0:gn, 1, :].rearrange("p g (h w) -> p g h w", h=H),
                in1=p3[:, 0:gn, :, None].to_broadcast([P, gn, H, c]),
                op=mybir.AluOpType.mult)
            nc.vector.tensor_copy(out=pv4[:, 0:gn, :, c], in_=p3[:, 0:gn, :])
            oh = pool.tile([P, G * W], BF, tag="oh")
            nc.vector.tensor_tensor(
                out=oh[:].rearrange("p (g f) -> p g f", g=G)[:, 0:gn],
                in0=iota[:, None, :].to_broadcast([P, gn, W]),
                in1=dstrel_sb[:, pr * nck + g0: pr * nck + g0 + gn][:, :, None]
                    .to_broadcast([P, gn, W]),
                op=mybir.AluOpType.is_equal)
            for j in range(gn):
                par, _ = ck_map[g0 + j]
                b0 = bands[g0 + j]
                nleft[par] -= 1
                rhs_pv = pv[:, j * H * (c + 1):(j + 1) * H * (c + 1)]
                if b0 == 32:
                    # PE stripe rule: base 32 allows only 32 partitions
                    nc.tensor.matmul(
                        out=aggs[par][32:64, :],
                        lhsT=oh[:, j * W:j * W + 32],
                        rhs=rhs_pv, start=False, stop=False)
                    nc.tensor.matmul(
                        out=aggs[par][64:96, :],
                        lhsT=oh[:, j * W + 32:(j + 1) * W],
                        rhs=rhs_pv, start=False, stop=nleft[par] == 0)
                else:
                    nc.tensor.matmul(
                        out=aggs[par][b0:b0 + W, :],
                        lhsT=oh[:, j * W:(j + 1) * W],
                        rhs=rhs_pv, start=False, stop=nleft[par] == 0,
                    )

        # stash the pair's PSUM aggregates; epilogues batched per TG tiles
        for i in range(GS):
            t = pr * GS + i
            tg = t % TG
            if tg == 0:
                agg_grp = pool.tile([P, TG * H * (c + 1)], FP, tag="agg_grp")
            nc.vector.tensor_copy(
                out=agg_grp[:, tg * H * (c + 1):(tg + 1) * H * (c + 1)],
                in_=aggs[i][:])
            if tg == TG - 1 or t == t_core - 1:
                n = tg + 1
                t0 = t - tg
                a4 = agg_grp[:].rearrange("p (t h w) -> p t h w", t=TG, h=H)
                rs = pool.tile([P, TG * H], FP, tag="rs")
                nc.vector.tensor_scalar(
                    out=rs[:, 0:n * H],
                    in0=a4[:, 0:n, :, c].rearrange("p t h -> p (t h)"),
                    scalar1=1e-30, scalar2=None, op0=mybir.AluOpType.add)
                nc.vector.reciprocal(out=rs[:, 0:n * H], in_=rs[:, 0:n * H])
                nc.vector.tensor_scalar(
                    out=rs[:, 0:n * H], in0=rs[:, 0:n * H], scalar1=1.0 / H,
                    scalar2=None, op0=mybir.AluOpType.mult)
                nc.vector.tensor_tensor(
                    out=a4[:, 0:n, :, 0:c], in0=a4[:, 0:n, :, 0:c],
                    in1=rs[:].rearrange("p (t h) -> p t h", t=TG)[:, 0:n, :,
                                                                  None]
                        .to_broadcast([P, n, H, c]),
                    op=mybir.AluOpType.mult)
                hsum = pool.tile([P, TG * c], FP, tag="hsum")
                nc.vector.reduce_sum(
                    out=hsum[:].rearrange("p (t w) -> p t w", t=TG)[:, 0:n],
                    in_=agg_grp[:].rearrange("p (t h w) -> p t w h", t=TG,
                                             h=H)[:, 0:n, 0:c, :],
                    axis=mybir.AxisListType.X)
                # 1/H already folded into rs; skip holds bs + mean_h bv
                nc.vector.tensor_tensor(
                    out=hsum[:, 0:n * c], in0=hsum[:, 0:n * c],
                    in1=skip_all[:, t0 * c:(t0 + n) * c],
                    op=mybir.AluOpType.add)
                nc.scalar.activation(
                    out=h_res[:, t0 * c:(t0 + n) * c], in_=hsum[:, 0:n * c],
                    func=mybir.ActivationFunctionType.Relu)


def build_device(dcfg):
    t_core, cl, ch = dcfg["t_core"], dcfg["cl"], dcfg["ch"]
    ct = cl + ch
    n_pad, n_core = dcfg["n_pad"], dcfg["n_core"]
    fn, fe, H = dcfg["fn"], dcfg["fe"], dcfg["h"]
    c1, c2, ncls = dcfg["c1"], dcfg["c2"], dcfg["ncls"]
    hc1, hc2 = H * c1, H * c2
    hid = 2 * c2
    t_all = n_pad // P
    reps = int(os.environ.get("KBUILD_REPS", "1"))

    nc = bacc.Bacc("TRN2", target_bir_lowering=False, debug=False,
                   num_devices=NCORES, num_swdge_queues=4)

    def param(name, shape, dtype=FP, out=False):
        return nc.declare_dram_parameter(name, list(shape), dtype, isOutput=out)

    xT_d = param("xT", [fn, n_pad], BF)
    xT_own_d = param("xT_own", [fn, n_core], BF)
    eaT_d = param("eaT", [t_core // GS, fe, GS * ct * P], BF)
    kvidx_d = param("kvidx", [P, t_core * ct * 8], mybir.dt.int16)
    qidx_d = param("qidx", [P, t_core * ct * 8], mybir.dt.int16)
    dstrel_d = param("dstrel", [P, t_core * ct], BF)
    wkv1_d = param("wkv1", [fn, 2 * hc1], BF)
    wqs1_d = param("wqs1", [fn, hc1 + c1], BF)
    bqs1_d = param("bqs1", [1, hc1 + c1], BF)
    we1_d = param("we1", [fe, hc1], BF)
    wkv2_d = param("wkv2", [c1, 2 * hc2], BF)
    wqs2_d = param("wqs2", [c1, hc2 + c2], BF)
    bqs2_d = param("bqs2", [1, hc2 + c2], BF)
    we2_d = param("we2", [fe, hc2], BF)
    w3_d = param("w3", [c2, hid], BF)
    b3_d = param("b3", [hid, 1], FP)
    w4_d = param("w4", [hid, ncls], BF)
    b4_d = param("b4", [ncls, 1], FP)
    out_d = param("out", [ncls, n_core], out=True)

    with tile.TileContext(nc) as tc:
        with (
            tc.tile_pool(name="res", bufs=1) as res,
            tc.tile_pool(name="dram", bufs=1, space="DRAM") as dram,
        ):
            # ---- constants / resident tensors
            ident = res.tile([P, P], FP)
            make_identity(nc, ident[:])
            W = dcfg["W"]
            iota = res.tile([P, W], BF)
            nc.gpsimd.iota(iota[:], pattern=[[1, W]], base=0,
                           channel_multiplier=0,
                           allow_small_or_imprecise_dtypes=True)
            zrow = res.tile([1, 192], BF)
            nc.vector.memset(zrow[:], 0.0)
            kvidx_sb = res.tile([P, t_core * ct * 8], mybir.dt.int16)
            nc.sync.dma_start(out=kvidx_sb[:], in_=kvidx_d[:])
            qidx_sb = res.tile([P, t_core * ct * 8], mybir.dt.int16)
            nc.sync.dma_start(out=qidx_sb[:], in_=qidx_d[:])
            dstrel_sb = res.tile([P, t_core * ct], BF)
            nc.sync.dma_start(out=dstrel_sb[:], in_=dstrel_d[:])

            def load_w(d, shape, tag, dt=BF):
                t = res.tile(list(shape), dt, tag=tag)
                nc.sync.dma_start(out=t[:], in_=d[:])
                return t

            wkv1 = load_w(wkv1_d, [fn, 2 * hc1], "wkv1")
            wqs1 = load_w(wqs1_d, [fn, hc1 + c1], "wqs1")
            bqs1 = load_w(bqs1_d, [1, hc1 + c1], "bqs1")
            we1 = load_w(we1_d, [fe, hc1], "we1")
            wkv2 = load_w(wkv2_d, [c1, 2 * hc2], "wkv2")
            wqs2 = load_w(wqs2_d, [c1, hc2 + c2], "wqs2")
            bqs2 = load_w(bqs2_d, [1, hc2 + c2], "bqs2")
            we2 = load_w(we2_d, [fe, hc2], "we2")
            w3 = load_w(w3_d, [c2, hid], "w3")
            b3 = load_w(b3_d, [hid, 1], "b3", FP)
            w4 = load_w(w4_d, [hid, ncls], "w4")
            b4 = load_w(b4_d, [ncls, 1], "b4", FP)
            ones_row = res.tile([1, P], BF)
            nc.vector.memset(ones_row[:], 1.0)

            skip1_all = res.tile([P, t_core * c1], FP)
            skip2_all = res.tile([P, t_core * c2], FP)
            h1_res = res.tile([P, t_core * c1], FP)
            h1T_bf = res.tile([c1, t_core * P], BF)
            h2_res = res.tile([P, t_core * c2], FP)
            h2T_res = res.tile([c2, t_core * P], BF)

            # ---- internal DRAM
            kv1_full = dram.tile([n_pad, 2 * hc1], BF)
            q1_dram = dram.tile([n_core, hc1], BF)
            h1T_shard = dram.tile([t_core, c1 * P], BF)
            h1T_full = dram.tile([t_all, c1 * P], BF)
            kv2_full = dram.tile([n_pad, 2 * hc2], BF)
            q2_dram = dram.tile([n_core, 2 * hc2], BF)   # q2 padded to 256B

            def emit_pipeline():
                # ---- phase A: full kv1 projection (all tiles, redundant per
                # core) + own q1/skip1
                XB = 8                                   # xT tiles per DMA
                if os.environ.get("KB_SKIP_A"):
                    pass
                else:
                 with (tc.tile_pool(name="psumA", bufs=4, space="PSUM") as psum,
                      tc.tile_pool(name="poolA", bufs=3) as pool):
                  for tb in range(0, t_all, XB):
                      xT_blk = pool.tile([fn, XB * P], BF, tag="xT_blk")
                      nc.sync.dma_start(
                          out=xT_blk[:], in_=xT_d[:, tb * P:(tb + XB) * P])
                      kv_stage = pool.tile([P, XB * 2 * hc1], BF, tag="kv_st")
                      for i in range(XB):
                          pr_ps = psum.tile([P, 2 * hc1], FP, space="PSUM",
                                            tag="pr_ps")
                          nc.tensor.matmul(out=pr_ps[:],
                                           lhsT=xT_blk[:, i * P:(i + 1) * P],
                                           rhs=wkv1[:], start=True, stop=True)
                          if i % 2 == 0:
                              nc.scalar.activation(
                                  out=kv_stage[:, i * 2 * hc1:(i + 1) * 2 * hc1],
                                  in_=pr_ps[:],
                                  func=mybir.ActivationFunctionType.Copy)
                          else:
                              nc.vector.tensor_copy(
                                  out=kv_stage[:, i * 2 * hc1:(i + 1) * 2 * hc1],
                                  in_=pr_ps[:])
                      nc.sync.dma_start(
                          out=kv1_full[tb * P:(tb + XB) * P, :].rearrange(
                              "(t p) w -> p t w", p=P),
                          in_=kv_stage[:].rearrange("p (t w) -> p t w", t=XB))

                  # own q1/skip1 projections (bias row folded: [bq1 | bs1'])
                  QB = 8
                  for tb in range(0, t_core, QB):
                      bn = min(QB, t_core - tb)
                      xT_blk = pool.tile([fn, QB * P], BF, tag="xTq_blk")
                      nc.sync.dma_start(
                          out=xT_blk[:, 0:bn * P],
                          in_=xT_own_d[:, tb * P:(tb + bn) * P])
                      q_stage = pool.tile([P, QB * hc1], BF, tag="q_st")
                      for i in range(bn):
                          t = tb + i
                          qs_ps = psum.tile([P, hc1 + c1], FP, space="PSUM",
                                            tag="qs_ps")
                          nc.tensor.matmul(out=qs_ps[:],
                                           lhsT=xT_blk[:, i * P:(i + 1) * P],
                                           rhs=wqs1[:], start=True, stop=False)
                          nc.tensor.matmul(out=qs_ps[:], lhsT=ones_row[:1, :],
                                           rhs=bqs1[:1, :], start=False,
                                           stop=True)
                          nc.scalar.activation(
                              out=q_stage[:, i * hc1:(i + 1) * hc1],
                              in_=qs_ps[:, 0:hc1],
                              func=mybir.ActivationFunctionType.Copy)
                          nc.vector.tensor_copy(
                              out=skip1_all[:, t * c1:(t + 1) * c1],
                              in_=qs_ps[:, hc1:])
                      nc.sync.dma_start(
                          out=q1_dram[tb * P:(tb + bn) * P, :].rearrange(
                              "(t p) w -> p t w", p=P),
                          in_=q_stage[:, 0:bn * hc1].rearrange(
                              "p (t w) -> p t w", t=bn))

                consts = dict(iota=iota, zrow=zrow, kvidx=kvidx_sb,
                              qidx=qidx_sb, dstrel=dstrel_sb)

                # ---- phase B: layer-1 edge pass
                if os.environ.get("KB_SKIP_E1"):
                    nc.vector.memset(h1_res[:], 0.0)
                else:
                  with (tc.tile_pool(name="psumB", bufs=3, space="PSUM") as psum,
                        tc.tile_pool(name="poolB", bufs=3) as pool):
                    _edge_layer(nc, tc, pool, psum, dcfg, consts, dict(
                        c=c1, q_dram=q1_dram, qe_w=hc1, kv_full=kv1_full,
                        We_sb=we1, eaT_dram=eaT_d, skip_all=skip1_all[:],
                        h_res=h1_res[:]))

                # ---- phase C: transpose own h1 -> bf16, ship to allgather
                with tc.tile_pool(name="psumC", bufs=2, space="PSUM") as psum:
                    for t in range(t_core):
                        h1T_ps = psum.tile([c1, P], FP, space="PSUM",
                                           tag="h1T_ps")
                        nc.tensor.transpose(
                            out=h1T_ps[:], in_=h1_res[:, t * c1:(t + 1) * c1],
                            identity=ident[:])
                        eng = nc.scalar if t % 2 == 0 else nc.vector
                        if t % 2 == 0:
                            nc.scalar.activation(
                                out=h1T_bf[:, t * P:(t + 1) * P],
                                in_=h1T_ps[:],
                                func=mybir.ActivationFunctionType.Copy)
                        else:
                            nc.vector.tensor_copy(
                                out=h1T_bf[:, t * P:(t + 1) * P],
                                in_=h1T_ps[:])
                    nc.sync.dma_start(
                        out=h1T_shard[:].rearrange("t (c p) -> c t p", c=c1),
                        in_=h1T_bf[:].rearrange("c (t p) -> c t p", t=t_core))

                if not os.environ.get("KB_SKIP_AG"):
                    nc.gpsimd.collective_compute(
                        "AllGather", mybir.AluOpType.bypass,
                        replica_groups=[list(range(NCORES))],
                        ins=[h1T_shard[:].opt()], outs=[h1T_full[:].opt()])

                # ---- phase D: full kv2 projection + own q2/skip2
                HB = 8
                if os.environ.get("KB_SKIP_D"):
                    pass
                else:
                 with (tc.tile_pool(name="psumD", bufs=4, space="PSUM") as psum,
                      tc.tile_pool(name="poolD", bufs=3) as pool):
                  for tb in range(0, t_all, HB):
                      hT_blk = pool.tile([c1, HB * P], BF, tag="hT_blk")
                      nc.sync.dma_start(
                          out=hT_blk[:].rearrange("c (t p) -> c t p", t=HB),
                          in_=h1T_full[tb:tb + HB].rearrange(
                              "t (c p) -> c t p", c=c1))
                      kv_stage = pool.tile([P, HB * 2 * hc2], BF, tag="kv2_st")
                      for i in range(HB):
                          pr_ps = psum.tile([P, 2 * hc2], FP, space="PSUM",
                                            tag="p2_ps")
                          nc.tensor.matmul(out=pr_ps[:],
                                           lhsT=hT_blk[:, i * P:(i + 1) * P],
                                           rhs=wkv2[:], start=True, stop=True)
                          if i % 2 == 0:
                              nc.scalar.activation(
                                  out=kv_stage[:, i * 2 * hc2:(i + 1) * 2 * hc2],
                                  in_=pr_ps[:],
                                  func=mybir.ActivationFunctionType.Copy)
                          else:
                              nc.vector.tensor_copy(
                                  out=kv_stage[:, i * 2 * hc2:(i + 1) * 2 * hc2],
                                  in_=pr_ps[:])
                      nc.sync.dma_start(
                          out=kv2_full[tb * P:(tb + HB) * P, :].rearrange(
                              "(t p) w -> p t w", p=P),
                          in_=kv_stage[:].rearrange("p (t w) -> p t w", t=HB))

                  QB = 8
                  for tb in range(0, t_core, QB):
                      bn = min(QB, t_core - tb)
                      q_stage = pool.tile([P, QB * 2 * hc2], BF, tag="q2_st")
                      if tb < 2 * QB:
                          nc.vector.memset(
                              q_stage[:].rearrange(
                                  "p (t w) -> p t w", t=QB)[:, :, hc2:],
                              0.0)
                      for i in range(bn):
                          t = tb + i
                          qs_ps = psum.tile([P, hc2 + c2], FP, space="PSUM",
                                            tag="q2s_ps")
                          nc.tensor.matmul(
                              out=qs_ps[:],
                              lhsT=h1T_bf[:, t * P:(t + 1) * P],
                              rhs=wqs2[:], start=True, stop=False)
                          nc.tensor.matmul(out=qs_ps[:], lhsT=ones_row[:1, :],
                                           rhs=bqs2[:1, :], start=False,
                                           stop=True)
                          nc.scalar.activation(
                              out=q_stage[:, i * 2 * hc2:i * 2 * hc2 + hc2],
                              in_=qs_ps[:, 0:hc2],
                              func=mybir.ActivationFunctionType.Copy)
                          nc.vector.tensor_copy(
                              out=skip2_all[:, t * c2:(t + 1) * c2],
                              in_=qs_ps[:, hc2:])
                      nc.sync.dma_start(
                          out=q2_dram[tb * P:(tb + bn) * P, :].rearrange(
                              "(t p) w -> p t w", p=P),
                          in_=q_stage[:, 0:bn * 2 * hc2].rearrange(
                              "p (t w) -> p t w", t=bn))

                # ---- phase E: layer-2 edge pass
                with (tc.tile_pool(name="psumE", bufs=3, space="PSUM") as psum,
                      tc.tile_pool(name="poolE", bufs=3) as pool):
                    if os.environ.get("KB_SKIP_E2"):
                        nc.vector.memset(h2_res[:], 0.0)
                    else:
                      _edge_layer(nc, tc, pool, psum, dcfg, consts, dict(
                        c=c2, q_dram=q2_dram, qe_w=2 * hc2, kv_full=kv2_full,
                        We_sb=we2, eaT_dram=eaT_d, skip_all=skip2_all[:],
                        h_res=h2_res[:]))
                    for t in range(t_core):
                        h2T_ps = psum.tile([c2, P], FP, space="PSUM",
                                           tag="h2T_ps")
                        nc.tensor.transpose(
                            out=h2T_ps[:], in_=h2_res[:, t * c2:(t + 1) * c2],
                            identity=ident[:])
                        nc.vector.tensor_copy(
                            out=h2T_res[:, t * P:(t + 1) * P], in_=h2T_ps[:])

                # ---- phase F: dense head (output transposed [ncls, n_core])
                CHUNK = 512
                with (tc.tile_pool(name="psumF", bufs=2, space="PSUM") as psum,
                      tc.tile_pool(name="poolF", bufs=2) as pool):
                  for k0 in range(0, n_core, CHUNK):
                      kn = min(CHUNK, n_core - k0)
                      h3_ps = psum.tile([hid, CHUNK], FP, space="PSUM",
                                        tag="h3_ps")
                      nc.tensor.matmul(out=h3_ps[:, 0:kn], lhsT=w3[:],
                                       rhs=h2T_res[:, k0:k0 + kn], start=True,
                                       stop=True)
                      h3_sb = pool.tile([hid, CHUNK], BF, tag="h3_sb")
                      nc.scalar.activation(
                          out=h3_sb[:, 0:kn], in_=h3_ps[:, 0:kn],
                          func=mybir.ActivationFunctionType.Relu,
                          bias=b3[:, 0:1])
                      o_ps = psum.tile([ncls, CHUNK], FP, space="PSUM",
                                       tag="o_ps")
                      nc.tensor.matmul(out=o_ps[:, 0:kn], lhsT=w4[:],
                                       rhs=h3_sb[:, 0:kn], start=True,
                                       stop=True)
                      o_sb = pool.tile([ncls, CHUNK], FP, tag="o_sb")
                      nc.vector.tensor_scalar(
                          out=o_sb[:, 0:kn], in0=o_ps[:, 0:kn],
                          scalar1=b4[:, 0:1],
                          scalar2=None, op0=mybir.AluOpType.add)
                      nc.sync.dma_start(out=out_d[:, k0:k0 + kn],
                                        in_=o_sb[:, 0:kn])

            for _rep in range(reps):
                emit_pipeline()

    nc.compile()
    return nc


# ----------------------------------------------------------------------------
# entry point
# ----------------------------------------------------------------------------

_CACHE = {}


def _get_nc(dcfg):
    key = (tuple(sorted(dcfg.items())), os.environ.get("KBUILD_REPS", "1"))
    if key not in _CACHE:
        _CACHE[key] = build_device(dcfg)
    return _CACHE[key]


def prepare_in_maps(inputs):
    x = np.asarray(inputs["x"], np.float32)
    n_nodes = x.shape[0]
    n_edges = np.asarray(inputs["edge_index"]).shape[1]
    percore, dcfg = host_prep(
        x, np.asarray(inputs["edge_index"]),
        np.asarray(inputs["edge_attr"], np.float32),
        n_nodes, n_edges, np.asarray(inputs["edge_attr"]).shape[1])
    bf = lambda a: np.ascontiguousarray(
        np.asarray(a, np.float32).astype(ml_dtypes.bfloat16))
    f32 = lambda a: np.ascontiguousarray(np.asarray(a, np.float32))
    i = inputs
    H = HEADS
    # fold mean-over-heads of bv into the skip bias (sum_e alpha = 1)
    bs1f = (np.asarray(i["bs1"], np.float32)
            + np.asarray(i["bv1"], np.float32).reshape(H, C1).mean(0))
    bs2f = (np.asarray(i["bs2"], np.float32)
            + np.asarray(i["bv2"], np.float32).reshape(H, C2).mean(0))
    weights = dict(
        wkv1=bf(np.concatenate([f32(i["Wk1"]), f32(i["Wv1"])], axis=1)),
        wqs1=bf(np.concatenate([f32(i["Wq1"]), f32(i["Ws1"])], axis=1)),
        bqs1=bf(np.concatenate([f32(i["bq1"]), bs1f])[None, :]),
        we1=bf(i["We1"]),
        wkv2=bf(np.concatenate([f32(i["Wk2"]), f32(i["Wv2"])], axis=1)),
        wqs2=bf(np.concatenate([f32(i["Wq2"]), f32(i["Ws2"])], axis=1)),
        bqs2=bf(np.concatenate([f32(i["bq2"]), bs2f])[None, :]),
        we2=bf(i["We2"]),
        w3=bf(i["W3"]), b3=f32(i["b3"])[:, None],
        w4=bf(i["W4"]), b4=f32(i["b4"])[:, None],
    )
    return [dict(pc, **weights) for pc in percore], dcfg


def assemble_output(res, inputs):
    n_nodes = np.asarray(inputs["x"]).shape[0]
    out = np.concatenate([res.results[i]["out"].T for i in range(NCORES)])
    return np.ascontiguousarray(out[:n_nodes])


def kernel(x, edge_index, edge_attr,
           Wq1, bq1, Wk1, bk1, Wv1, bv1, We1, Ws1, bs1,
           Wq2, bq2, Wk2, bk2, Wv2, bv2, We2, Ws2, bs2,
           W3, b3, W4, b4):
    inputs = dict(
        x=x, edge_index=edge_index, edge_attr=edge_attr,
        Wq1=Wq1, bq1=bq1, Wk1=Wk1, bk1=bk1, Wv1=Wv1, bv1=bv1, We1=We1,
        Ws1=Ws1, bs1=bs1,
        Wq2=Wq2, bq2=bq2, Wk2=Wk2, bk2=bk2, Wv2=Wv2, bv2=bv2, We2=We2,
        Ws2=Ws2, bs2=bs2,
        W3=W3, b3=b3, W4=W4, b4=b4,
    )
    in_maps, dcfg = prepare_in_maps(inputs)
    nc = _get_nc(dcfg)
    res = run_bass_kernel_spmd(nc, in_maps, core_ids=list(range(NCORES)))
    return assemble_output(res, inputs)
